# revision 1
# baseline (speedup 1.0000x reference)
"""Trainium2 Bass kernel for nn_BidirectionalLayerFeatCosine (retrieval_knn).

Strategy: shard the 4096 query points across 8 NeuronCores (512 each); keys
are replicated.  Each core runs an identical SPMD program over 4 combos
(2 crosses x 2 batches):
  - normalize key/query knn features (cosine), build euclid augmented tensors
  - PE matmuls produce cosine / euclid score rows [128q x 4096k]
  - DVE max8 + find_index8 give exact top-8 neighbor indices per metric
  - GpSimd ap_gather pulls neighbor features (pos-conv folded in linearly)
  - PE 64x64 MLP layers + ACT leaky-relu, GpSimd max-pool over 16 neighbors
"""
import sys

for _p in ('/opt/trn_rl_repo',):
    if _p not in sys.path:
        sys.path.insert(0, _p)

import numpy as np
import concourse.bass as bass
import concourse.tile as tile
from concourse import bacc, mybir

F32 = mybir.dt.float32
I16 = mybir.dt.int16
U16 = mybir.dt.uint16
AF = mybir.ActivationFunctionType
ALU = mybir.AluOpType

B, N, C, NS = 2, 4096, 64, 16
NCORES = 8
LEAKY = 0.1
EPS = 1e-8


def _neg_sq_row(tc, pools, pc_ap, length, blk3, id128, dram_pool, dst_row_ap):
    """Write -(x^2+y^2+z^2) per point of pc_ap [3, length] (DRAM) into
    dst_row_ap [1, length] via a stacked-transpose matmul-sum + DRAM bounce."""
    nc = tc.nc
    nbs = length // 128          # stacked blocks of 128 cols
    assert length % 128 == 0 and 3 * nbs <= 128
    small = pools['small']
    tp = pools['tpose_ps']

    xst = small.tile([3 * nbs, 128], F32, tag='xst')
    # partition 3*b + c  <-  pc[c, 128b : 128(b+1)]
    src = bass.AP(pc_ap.tensor, pc_ap.offset,
                  [[128, nbs], [length, 3], [1, 128]])
    nc.sync.dma_start(xst[:], src)
    xsq = small.tile([3 * nbs, 128], F32, tag='xsq')
    nc.scalar.activation(xsq[:], xst[:], AF.Square)
    # out[m, b] = sum_c xsq[3b+c, m]
    ps = tp.tile([128, 128], F32, tag='tp')
    nc.tensor.matmul(ps[:, :nbs], lhsT=xsq[:], rhs=blk3[:3 * nbs, :nbs],
                     start=True, stop=True)
    sq_sb = small.tile([128, nbs], F32, tag='sq_sb')
    nc.scalar.activation(sq_sb[:], ps[:, :nbs], AF.Copy)
    # transpose [128, nbs] -> [nbs, 128], negate on copy-out
    ps2 = tp.tile([128, 128], F32, tag='tp')
    nc.tensor.matmul(ps2[:nbs, :], lhsT=sq_sb[:], rhs=id128[:],
                     start=True, stop=True)
    negsq = small.tile([nbs, 128], F32, tag='negsq')
    nc.scalar.activation(negsq[:], ps2[:nbs, :], AF.Copy, scale=-1.0)
    # bounce through DRAM: [nbs, 128] row-major == flat [length]
    scratch = dram_pool.tile([nbs, 128], F32, tag='nsq_scratch')
    nc.sync.dma_start(scratch[:], negsq[:])
    if not isinstance(dst_row_ap, list):
        dst_row_ap = [(dst_row_ap, 0, length)]
    for dst, coff, w in dst_row_ap:
        flat = bass.AP(scratch[:].tensor, scratch[:].offset + coff,
                       [[w, 1], [1, w]])
        nc.sync.dma_start(dst, flat)


def _neg_sq_row_rep(tc, pools, pc_ap, length, blk3, id128, dram_pool,
                    dst_2rows):
    """Like _neg_sq_row but dst is [2, length] (same data both rows)."""
    # write via a [2, length]-iterating DMA from the same flat scratch twice
    nc = tc.nc
    small = pools['small']
    tp = pools['tpose_ps']
    nbs = length // 128
    xst = small.tile([3 * nbs, 128], F32, tag='xst')
    src = bass.AP(pc_ap.tensor, pc_ap.offset,
                  [[128, nbs], [length, 3], [1, 128]])
    nc.sync.dma_start(xst[:], src)
    xsq = small.tile([3 * nbs, 128], F32, tag='xsq')
    nc.scalar.activation(xsq[:], xst[:], AF.Square)
    ps = tp.tile([128, 128], F32, tag='tp')
    nc.tensor.matmul(ps[:, :nbs], lhsT=xsq[:], rhs=blk3[:3 * nbs, :nbs],
                     start=True, stop=True)
    sq_sb = small.tile([128, nbs], F32, tag='sq_sb')
    nc.scalar.activation(sq_sb[:], ps[:, :nbs], AF.Copy)
    ps2 = tp.tile([128, 128], F32, tag='tp')
    nc.tensor.matmul(ps2[:nbs, :], lhsT=sq_sb[:], rhs=id128[:],
                     start=True, stop=True)
    negsq = small.tile([nbs, 128], F32, tag='negsq')
    nc.scalar.activation(negsq[:], ps2[:nbs, :], AF.Copy, scale=-1.0)
    scratch = dram_pool.tile([nbs, 128], F32, tag='nsq_scratch')
    nc.sync.dma_start(scratch[:], negsq[:])
    flat2 = bass.AP(scratch[:].tensor, scratch[:].offset,
                    [[0, 2], [1, length]])
    nc.sync.dma_start(dst_2rows, flat2)


def _normalize_cm(tc, pools, src_getter, length, id128, out_tile, eps128=None):
    """L2-normalize columns of a channel-major [64, length] tensor.
    src_getter(blk) -> AP [64, 128] (SBUF) for column block blk.
    Writes normalized columns into out_tile [64, length]."""
    nc = tc.nc
    nb = length // 128
    small = pools['small']
    tp = pools['tpose_ps']

    ss = small.tile([128, nb], F32, tag='ss')
    # sweep 1: per-block transpose + square-accumulate row sums
    for blk in range(nb):
        ps = tp.tile([128, 128], F32, tag='tp')
        nc.tensor.matmul(ps[:, :C], lhsT=src_getter(blk), rhs=id128[:C, :C],
                         start=True, stop=True)
        sq = small.tile([128, C], F32, tag='sqscratch')
        nc.scalar.activation(sq[:], ps[:, :C], AF.Square,
                             accum_out=ss[:, blk:blk + 1])
    # rinv = 1 / sqrt(ss + eps)   (ACT sqrt, accurate recip on DVE)
    nrm = small.tile([128, nb], F32, tag='nrm')
    nc.scalar.activation(nrm[:], ss[:], AF.Sqrt, bias=eps128[:])
    rinv = small.tile([128, nb], F32, tag='rinv')
    nc.vector.reciprocal(rinv[:], nrm[:])
    # sweep 2: re-transpose, scale by rinv, transpose back
    for blk in range(nb):
        ps = tp.tile([128, 128], F32, tag='tp')
        nc.tensor.matmul(ps[:, :C], lhsT=src_getter(blk), rhs=id128[:C, :C],
                         start=True, stop=True)
        pm = small.tile([128, C], F32, tag='pmscaled')
        nc.scalar.activation(pm[:], ps[:, :C], AF.Copy,
                             scale=rinv[:, blk:blk + 1])
        ps2 = tp.tile([128, 128], F32, tag='tp')
        nc.tensor.matmul(ps2[:C, :], lhsT=pm[:], rhs=id128[:], start=True,
                         stop=True)
        nc.scalar.activation(out_tile[:, blk * 128:(blk + 1) * 128],
                             ps2[:C, :], AF.Copy)


def build_nc(Q=N // NCORES, NK=N, debug_taps=False):
    """Build the SPMD Bass program.  Q = queries per core per combo,
    NK = number of keys."""
    nc = bacc.Bacc("TRN2", num_devices=NCORES, debug=False)

    def din(name, shape):
        return nc.dram_tensor(name, list(shape), F32, kind="ExternalInput").ap()

    ins = {}
    for nm, sh in [
        ('knn1f', (B, C, NK)), ('knn2f', (B, C, NK)),
        ('feat1f', (B, C, NK)), ('feat2f', (B, C, NK)),
        ('pc1f', (B, 3, NK)), ('pc2f', (B, 3, NK)),
        ('knn1q', (B, C, Q)), ('knn2q', (B, C, Q)),
        ('feat1q', (B, C, Q)), ('feat2q', (B, C, Q)),
        ('pc1q', (B, 3, Q)), ('pc2q', (B, 3, Q)),
        ('wt11T', (C, C)), ('wt22T', (C, C)), ('wposT', (3, C)),
        ('wm1T', (C, C)), ('wm2T', (C, C)),
        ('b11', (C, 1)), ('b22', (C, 1)), ('bpos', (C, 1)),
        ('bm1', (C, 1)), ('bm2', (C, 1)),
        ('id128', (128, 128)), ('blk3', (128, 128)),
        ('ones_row', (1, max(NK, Q))),
    ]:
        ins[nm] = din(nm, sh)
    out1 = nc.dram_tensor('out1', [B, C, Q], F32, kind="ExternalOutput").ap()
    out2 = nc.dram_tensor('out2', [B, C, Q], F32, kind="ExternalOutput").ap()
    taps = None
    if debug_taps:
        taps = {nm: nc.dram_tensor(nm, list(sh), dt, kind="ExternalOutput").ap()
                for nm, sh, dt in [
                    ('dbg_idx', (128, 16), F32),
                    ('dbg_ag', (C, 128 * NS), F32),
                    ('dbg_n1', (C, 128 * NS), F32),
                    ('dbg_h1', (C, 128 * NS), F32),
                    ('dbg_h2', (C, 128 * NS), F32),
                    ('dbg_khat', (C, NK), F32),
                    ('dbg_cq', (C, Q), F32),
                    ('dbg_akv', (C, NK), F32),
                ]}

    with tile.TileContext(nc) as tc:
        _kernel_body(tc, ins, out1, out2, Q, NK, taps)
    nc.compile()
    return nc


def _kernel_body(tc, ins, out1, out2, Q, NK, taps=None):
    nc = tc.nc
    from contextlib import ExitStack
    ctx = ExitStack()
    NB = NK // 512      # key blocks for score matmuls
    NT = Q // 128       # query tiles
    NROW = 128 * NS     # gathered rows per tile (2048)

    pool = lambda name, bufs: ctx.enter_context(
        tc.tile_pool(name=name, bufs=bufs))
    consts = pool('consts', 1)
    small = pool('small', 3)
    khatp = pool('khat', 2)
    knnkvp = pool('knnkv', 1)
    qhatp = pool('qhat', 2)
    akvp = pool('akv', 1)
    augp = pool('aug', 1)
    qp = pool('qtensors', 1)
    cqp = pool('cq', 2)
    scoresp = pool('scores', 2)
    mlpp = pool('mlp', 4)
    mpp = pool('maxpool', 1)
    outp = pool('outtile', 2)
    tpose_ps = ctx.enter_context(
        tc.tile_pool(name='tpose_ps', bufs=4, space='PSUM'))
    score_ps = ctx.enter_context(
        tc.tile_pool(name='score_ps', bufs=2, space='PSUM'))
    mlp_ps = ctx.enter_context(
        tc.tile_pool(name='mlp_ps', bufs=2, space='PSUM'))
    dram_pool = ctx.enter_context(
        tc.tile_pool(name='dram', bufs=2, space='DRAM'))
    pools = {'small': small, 'tpose_ps': tpose_ps}

    # ---- load constants ----
    def cload(name, shape):
        t = consts.tile(list(shape), F32, tag=name)
        nc.sync.dma_start(t[:], ins[name])
        return t

    id128 = cload('id128', (128, 128))
    blk3 = cload('blk3', (128, 128))
    wt11T = cload('wt11T', (C, C))
    wt22T = cload('wt22T', (C, C))
    wposT = cload('wposT', (3, C))
    wm1T = cload('wm1T', (C, C))
    wm2T = cload('wm2T', (C, C))
    b11 = cload('b11', (C, 1))
    b22 = cload('b22', (C, 1))
    bpos = cload('bpos', (C, 1))
    bm1 = cload('bm1', (C, 1))
    bm2 = cload('bm2', (C, 1))
    # combined query-side bias b11 + bpos
    bqc = consts.tile([C, 1], F32, tag='bqc')
    nc.vector.tensor_add(bqc[:], b11[:], bpos[:])
    eps128 = consts.tile([128, 1], F32, tag='eps128')
    nc.vector.memset(eps128[:], EPS)

    combos = []
    for bi in range(B):
        combos.append((out1, bi, ins['knn1q'], ins['feat1q'], ins['pc1q'],
                       ins['knn2f'], ins['feat2f'], ins['pc2f']))
        combos.append((out2, bi, ins['knn2q'], ins['feat2q'], ins['pc2q'],
                       ins['knn1f'], ins['feat1f'], ins['pc1f']))

    for ci, (outap, bi, knnq_d, featq_d, pcq_d, knnf_d, featf_d, pcf_d) \
            in enumerate(combos):
        # ================= key-side prep =================
        # --- khat: normalized key knn features [64, NK] ---
        knnkv = knnkvp.tile([C, NK], F32, tag='knnkv')
        nc.sync.dma_start(knnkv[:], knnf_d[bi])
        khat = khatp.tile([C, NK], F32, tag='khat')
        _normalize_cm(tc, pools,
                      lambda blk: knnkv[:, blk * 128:(blk + 1) * 128],
                      NK, id128, khat, eps128=eps128)

        # --- A_kv = Wkv@feat_kv + b_kv + Wpos@xyz_kv  [64, NK] ---
        akv = akvp.tile([C, NK], F32, tag='akv')
        for kb in range(NB):
            sl = slice(kb * 512, (kb + 1) * 512)
            fb = qp.tile([C, 512], F32, tag='featblk')
            nc.sync.dma_start(fb[:], featf_d[bi][:, sl])
            xb = qp.tile([3, 512], F32, tag='xyzblk')
            nc.sync.dma_start(xb[:], pcf_d[bi][:, sl])
            ps = mlp_ps.tile([C, 512], F32, tag='mm')
            nc.tensor.matmul(ps[:], lhsT=wt22T[:], rhs=fb[:], start=True,
                             stop=False)
            nc.tensor.matmul(ps[:], lhsT=wposT[:], rhs=xb[:], start=False,
                             stop=True)
            nc.scalar.activation(akv[:, sl], ps[:], AF.Identity, bias=b22[:])

        # --- aug_kv: 2 half-stacks at base partitions 0 and 64 ---
        NQD = 2 if NK % 1024 == 0 else 1
        NK4 = NK // NQD
        augkv = augp.tile([64 * (NQD - 1) + 5, NK4], F32, tag='augkv')
        for qd in range(NQD):
            nc.sync.dma_start(augkv[64 * qd:64 * qd + 3, :],
                              pcf_d[bi][:, qd * NK4:(qd + 1) * NK4])
            nc.sync.dma_start(augkv[64 * qd + 4:64 * qd + 5, :],
                              ins['ones_row'][:, :NK4])
        dsts = [(augkv[64 * qd + 3:64 * qd + 4, :], qd * NK4, NK4)
                for qd in range(NQD)]
        _neg_sq_row(tc, pools, pcf_d[bi], NK, blk3, id128, dram_pool, dsts)

        # ================= query-side prep =================
        knnq = qp.tile([C, Q], F32, tag='knnq')
        nc.sync.dma_start(knnq[:], knnq_d[bi])
        qhat = qhatp.tile([C, Q], F32, tag='qhat')
        _normalize_cm(tc, pools,
                      lambda blk: knnq[:, blk * 128:(blk + 1) * 128],
                      Q, id128, qhat, eps128=eps128)

        xq = qp.tile([3, Q], F32, tag='xq')
        nc.sync.dma_start(xq[:], pcq_d[bi])
        fq = qp.tile([C, Q], F32, tag='fq')
        nc.sync.dma_start(fq[:], featq_d[bi])
        negxq = qp.tile([3, Q], F32, tag='negxq')
        nc.scalar.activation(negxq[:], xq[:], AF.Copy, scale=-1.0)
        # aug_q [2x, 2y, 2z, 1, -|q|^2], replicated at partitions 0 and 64
        augq = qp.tile([69, Q], F32, tag='augq')
        for qd in range(2):
            nc.scalar.activation(augq[64 * qd:64 * qd + 3, :], xq[:],
                                 AF.Copy, scale=2.0)
            nc.sync.dma_start(augq[64 * qd + 3:64 * qd + 4, :],
                              ins['ones_row'][:, :Q])
        dsts = [(augq[64 * qd + 4:64 * qd + 5, :], 0, Q) for qd in range(2)]
        _neg_sq_row(tc, pools, pcq_d[bi], Q, blk3, id128, dram_pool, dsts)
        # C_q = Wq@feat_q + b11 - Wpos@xyz_q + bpos  [64, Q]
        cq = cqp.tile([C, Q], F32, tag='cq')
        for qb0 in range(0, Q, 512):
            w = min(512, Q - qb0)
            sl = slice(qb0, qb0 + w)
            cps = mlp_ps.tile([C, 512], F32, tag='mm')
            nc.tensor.matmul(cps[:, :w], lhsT=wt11T[:], rhs=fq[:, sl],
                             start=True, stop=False)
            nc.tensor.matmul(cps[:, :w], lhsT=wposT[:], rhs=negxq[:, sl],
                             start=False, stop=True)
            nc.scalar.activation(cq[:, sl], cps[:, :w], AF.Identity,
                                 bias=bqc[:])

        if taps is not None and ci == 0:
            nc.sync.dma_start(taps['dbg_khat'], khat[:])
            nc.sync.dma_start(taps['dbg_cq'], cq[:])
            nc.sync.dma_start(taps['dbg_akv'], akv[:])

        # ================= per query-tile =================
        for t in range(NT):
            tsl = slice(t * 128, (t + 1) * 128)
            # --- scores ---
            sc_cos = scoresp.tile([128, NK], F32, tag='sc')
            for kb in range(NB):
                sl = slice(kb * 512, (kb + 1) * 512)
                ps = score_ps.tile([128, 512], F32, tag='sc_ps')
                nc.tensor.matmul(ps[:], lhsT=qhat[:, tsl], rhs=khat[:, sl],
                                 start=True, stop=True)
                nc.scalar.activation(sc_cos[:, sl], ps[:], AF.Copy)
            sc_euc = scoresp.tile([128, NK], F32, tag='sc')
            for kb in range(NB):
                sl = slice(kb * 512, (kb + 1) * 512)
                ps = score_ps.tile([128, 512], F32, tag='sc_ps')
                qd = (kb * 512) // NK4
                coff = kb * 512 - qd * NK4
                nc.tensor.matmul(ps[:], lhsT=augq[64 * qd:64 * qd + 5, tsl],
                                 rhs=augkv[64 * qd:64 * qd + 5,
                                           coff:coff + 512],
                                 start=True, stop=True)
                nc.scalar.activation(sc_euc[:, sl], ps[:], AF.Copy)

            # --- top-8 per metric (DVE) ---
            vals = small.tile([128, 16], F32, tag='vals')
            idxu = small.tile([128, 16], U16, tag='idxu')
            nc.vector.max(vals[:, 0:8], sc_cos[:])
            nc.vector.max_index(idxu[:, 0:8], vals[:, 0:8], sc_cos[:])
            nc.vector.max(vals[:, 8:16], sc_euc[:])
            nc.vector.max_index(idxu[:, 8:16], vals[:, 8:16], sc_euc[:])

            # --- index transpose to gather layout ---
            idxf = small.tile([128, 16], F32, tag='idxf')
            nc.vector.tensor_copy(idxf[:], idxu[:])
            pst = tpose_ps.tile([128, 128], F32, tag='tp')
            nc.tensor.matmul(pst[:16, :], lhsT=idxf[:], rhs=id128[:],
                             start=True, stop=True)
            idxT = small.tile([C, 128], I16, tag='idxT')
            nc.scalar.activation(idxT[0:16, :], pst[:16, :], AF.Copy)
            # replicate down to all 4 gpsimd 16-partition groups via DMA
            nc.sync.dma_start(idxT[16:32, :], idxT[0:16, :])
            nc.sync.dma_start(idxT[32:64, :], idxT[0:32, :])

            # --- gather neighbors (GpSimd) ---
            ag = mlpp.tile([C, NROW], F32, tag='mlp')
            nc.gpsimd.ap_gather(ag[:], akv[:], idxT[:], channels=C,
                                num_elems=NK, d=1, num_idxs=NROW)

            # --- layer 0: add per-query C_q, leaky relu ---
            n0 = mlpp.tile([C, NROW], F32, tag='mlp')
            cq_b = cq[:, tsl].to_broadcast([C, 128, NS])
            nc.vector.tensor_tensor(
                n0[:].rearrange('c (q k) -> c q k', k=NS),
                ag[:].rearrange('c (q k) -> c q k', k=NS),
                cq_b, op=ALU.add)
            n1 = mlpp.tile([C, NROW], F32, tag='mlp')
            nc.scalar.activation(n1[:], n0[:], AF.Prelu, alpha=LEAKY)
            if taps is not None and ci == 0 and t == 0:
                nc.sync.dma_start(taps['dbg_idx'], idxf[:])
                nc.sync.dma_start(taps['dbg_ag'], ag[:])
                nc.sync.dma_start(taps['dbg_n1'], n1[:])

            # --- layers 1, 2 ---
            cur = n1
            for w, bias in ((wm1T, bm1), (wm2T, bm2)):
                h = mlpp.tile([C, NROW], F32, tag='mlp')
                for j in range(NROW // 512):
                    sl = slice(j * 512, (j + 1) * 512)
                    ps = mlp_ps.tile([C, 512], F32, tag='mm')
                    nc.tensor.matmul(ps[:], lhsT=w[:], rhs=cur[:, sl],
                                     start=True, stop=True)
                    nc.scalar.activation(h[:, sl], ps[:], AF.Prelu,
                                         bias=bias[:], alpha=LEAKY)
                if taps is not None and ci == 0 and t == 0:
                    nc.sync.dma_start(
                        taps['dbg_h1' if bias is bm1 else 'dbg_h2'], h[:])
                cur = h

            # --- max-pool over NS neighbors (GpSimd pair tree) ---
            width = NS
            while width > 1:
                w2 = width // 2
                if w2 == 1:
                    nxt = outp.tile([C, 128], F32, tag='ot')
                else:
                    nxt = mpp.tile([C, 128 * w2], F32, tag=f'mp{w2}')
                v = cur[:].rearrange('c (q w two) -> c q w two', two=2, w=w2)
                nc.vector.tensor_tensor(
                    nxt[:].rearrange('c (q w) -> c q w', w=w2),
                    v[:, :, :, 0], v[:, :, :, 1], op=ALU.max)
                cur = nxt
                width = w2

            nc.sync.dma_start(outap[bi][:, tsl], cur[:])

    ctx.close()


# ======================= host side =======================

_CACHED = {}


def _get_nc():
    if 'nc' not in _CACHED:
        _CACHED['nc'] = build_nc()
    return _CACHED['nc']


def make_in_maps(pc1, pc2, feat1, feat2, knn1, knn2,
                 W_t11, b_t11, W_t22, b_t22, W_pos, b_pos,
                 W_m1, b_m1, W_m2, b_m2, Q=N // NCORES, NK=N,
                 ncores=NCORES):
    f32 = np.float32
    base = {
        'knn1f': np.ascontiguousarray(knn1, f32),
        'knn2f': np.ascontiguousarray(knn2, f32),
        'feat1f': np.ascontiguousarray(feat1, f32),
        'feat2f': np.ascontiguousarray(feat2, f32),
        'pc1f': np.ascontiguousarray(pc1, f32),
        'pc2f': np.ascontiguousarray(pc2, f32),
        'wt11T': np.ascontiguousarray(np.asarray(W_t11).T, f32),
        'wt22T': np.ascontiguousarray(np.asarray(W_t22).T, f32),
        'wposT': np.ascontiguousarray(np.asarray(W_pos).T, f32),
        'wm1T': np.ascontiguousarray(np.asarray(W_m1).T, f32),
        'wm2T': np.ascontiguousarray(np.asarray(W_m2).T, f32),
        'b11': np.ascontiguousarray(np.asarray(b_t11).reshape(C, 1), f32),
        'b22': np.ascontiguousarray(np.asarray(b_t22).reshape(C, 1), f32),
        'bpos': np.ascontiguousarray(np.asarray(b_pos).reshape(C, 1), f32),
        'bm1': np.ascontiguousarray(np.asarray(b_m1).reshape(C, 1), f32),
        'bm2': np.ascontiguousarray(np.asarray(b_m2).reshape(C, 1), f32),
        'id128': np.eye(128, dtype=f32),
        'blk3': (np.arange(128)[:, None] // 3 == np.arange(128)[None, :]
                 ).astype(f32),
        'ones_row': np.ones((1, max(NK, Q)), f32),
    }
    in_maps = []
    for c in range(ncores):
        sl = slice(c * Q, (c + 1) * Q)
        m = dict(base)
        m['knn1q'] = np.ascontiguousarray(base['knn1f'][:, :, sl])
        m['knn2q'] = np.ascontiguousarray(base['knn2f'][:, :, sl])
        m['feat1q'] = np.ascontiguousarray(base['feat1f'][:, :, sl])
        m['feat2q'] = np.ascontiguousarray(base['feat2f'][:, :, sl])
        m['pc1q'] = np.ascontiguousarray(base['pc1f'][:, :, sl])
        m['pc2q'] = np.ascontiguousarray(base['pc2f'][:, :, sl])
        in_maps.append(m)
    return in_maps


def kernel(pc1, pc2, feat1, feat2, knn1, knn2,
           W_t11, b_t11, W_t22, b_t22, W_pos, b_pos,
           W_m1, b_m1, W_m2, b_m2):
    from concourse.bass_utils import run_bass_kernel_spmd
    nc = _get_nc()
    in_maps = make_in_maps(pc1, pc2, feat1, feat2, knn1, knn2,
                           W_t11, b_t11, W_t22, b_t22, W_pos, b_pos,
                           W_m1, b_m1, W_m2, b_m2)
    res = run_bass_kernel_spmd(nc, in_maps, core_ids=list(range(NCORES)))
    out1 = np.concatenate([res.results[c]['out1'] for c in range(NCORES)],
                          axis=2)
    out2 = np.concatenate([res.results[c]['out2'] for c in range(NCORES)],
                          axis=2)
    return out1, out2



# revision 40
# speedup vs baseline: 49.3340x; 49.3340x over previous
"""Trainium2 Bass kernel for nn_BidirectionalLayerFeatCosine (retrieval_knn).

Strategy: shard the 4096 query points across 8 NeuronCores (512 each); keys
are replicated.  Host rolls the key axis per core so each core's query block
is always columns 0:512 (SPMD-clean static slices).

Per core, per batch: load knn / [feat;pc] for both sides once; normalize the
full knn tensors (gpsimd partition_all_reduce + divide -> khat, exact fp32;
query side is a slice of khat), compute akv = W22@feat + Wpos@pc + b22 with a
single fp16 matmul per chunk (67-row fused contract), and |pc|^2 rows in the
same tile for the 4-row euclid contract.  Scores (cos + euclid) stay exact
fp32 on the PE (4 cyc/col); DVE max8/find_index8 give exact top-8 indices;
GpSimd ap_gather pulls neighbors in a 128-partition stacked layout; the MLP
runs in fp16 with block-diagonal stacked weights; maxpool tree on gpsimd.
"""
import sys

for _p in ('/opt/trn_rl_repo',):
    if _p not in sys.path:
        sys.path.insert(0, _p)

import numpy as np
import concourse.bass as bass
import concourse.tile as tile
from concourse import bacc, mybir, bass_isa

F32 = mybir.dt.float32
F16 = mybir.dt.float16
I16 = mybir.dt.int16
U16 = mybir.dt.uint16
AF = mybir.ActivationFunctionType
ALU = mybir.AluOpType
ROP = bass_isa.ReduceOp

B, N, C, NS = 2, 4096, 64, 16
NCORES = 8
Q = N // NCORES           # queries per core per combo (512)
NT = Q // 128             # query tiles per combo (4)
NB = N // 512             # 512-col key chunks (8)
LEAKY = 0.1
EPS = 1e-8


def build_nc(debug_taps=False):
    nc = bacc.Bacc("TRN2", num_devices=NCORES, debug=False)

    def din(name, shape, dt=F32):
        return nc.dram_tensor(name, list(shape), dt, kind="ExternalInput").ap()

    ins = {
        'knn1': din('knn1', (B, C, N)),
        'knn2': din('knn2', (B, C, N)),
        'fkv1': din('fkv1', (B, C + 3, N)),
        'fkv2': din('fkv2', (B, C + 3, N)),
        'w22pT': din('w22pT', (C + 3, C), F16),
        'w11pnT': din('w11pnT', (C + 3, C)),
        'wm1stk': din('wm1stk', (128, 128), F16),
        'wm2stk': din('wm2stk', (128, 128), F16),
        'b22': din('b22', (C, 1)),
        'bqc': din('bqc', (C, 1)),
        'bm1s': din('bm1s', (128, 1)),
        'bm2s': din('bm2s', (128, 1)),
        'id128': din('id128', (128, 128)),
    }
    out1 = nc.dram_tensor('out1', [B, C, Q], F32, kind="ExternalOutput").ap()
    out2 = nc.dram_tensor('out2', [B, C, Q], F32, kind="ExternalOutput").ap()
    taps = None
    if debug_taps:
        taps = {nm: nc.dram_tensor(nm, list(sh), dt, kind="ExternalOutput").ap()
                for nm, sh, dt in [
                    ('dbg_khat', (70, N), F32),
                    ('dbg_akv2', (128, N), F32),
                    ('dbg_cq', (C, Q), F32),
                    ('dbg_augq', (70, Q), F32),
                    ('dbg_sccos', (128, N), F32),
                    ('dbg_sceuc', (128, N), F32),
                    ('dbg_idxf', (128, 16), F32),
                    ('dbg_idxT', (128, 64), I16),
                    ('dbg_ag', (128, 1024), F32),
                    ('dbg_n1', (128, 1024), F16),
                    ('dbg_h2', (128, 1024), F16),
                ]}

    with tile.TileContext(nc) as tc:
        _body(tc, ins, out1, out2, taps)
    nc.compile()
    return nc


def _body(tc, ins, out1, out2, taps=None):
    nc = tc.nc
    from contextlib import ExitStack
    ctx = ExitStack()

    pool = lambda name, bufs, space='SBUF': ctx.enter_context(
        tc.tile_pool(name=name, bufs=bufs, space=space))

    consts = pool('consts', 1)
    inp = pool('inputs', 2)        # fkv tiles (rotate across sides)
    prep = pool('prep', 2)         # chunked scratch for normalization
    keyp = pool('keyprep', 2)      # khat / akv2 (both sides live)
    f16p = pool('f16', 1)
    qp = pool('qside', 2)          # cq / augq per combo
    scp = pool('scores', 2)        # [128, 4096] score rows
    idxp = pool('idx', 2)          # vals/idx tiles per tile
    mlpp = pool('mlp', 1)
    outp = pool('out', 2)

    sc_ps = ctx.enter_context(tc.tile_pool(name='sc_ps', bufs=2, space='PSUM'))
    mlp_ps = ctx.enter_context(tc.tile_pool(name='mlp_ps', bufs=1, space='PSUM'))
    p64_ps = ctx.enter_context(tc.tile_pool(name='p64_ps', bufs=1, space='PSUM'))
    tp_ps = ctx.enter_context(tc.tile_pool(name='tp_ps', bufs=1, space='PSUM'))

    # ---- constants ----
    def cload(name, shape, dt=F32):
        t = consts.tile(list(shape), dt, tag=name, name=name)
        nc.sync.dma_start(t[:], ins[name])
        return t

    w22pT = cload('w22pT', (C + 3, C), F16)
    w11pnT = cload('w11pnT', (C + 3, C))
    wm1stk = cload('wm1stk', (128, 128), F16)
    wm2stk = cload('wm2stk', (128, 128), F16)
    b22 = cload('b22', (C, 1))
    bqc = cload('bqc', (C, 1))
    bm1s = cload('bm1s', (128, 1))
    bm2s = cload('bm2s', (128, 1))
    id128 = cload('id128', (128, 128))
    eps64 = consts.tile([C, 1], F32, tag='eps64', name='eps64')
    nc.vector.memset(eps64[:], EPS)

    # ---------- per (batch, side) key prep ----------
    CH = 1024                      # normalization chunk width

    def key_prep(bi, side):
        knn_d = ins['knn1'] if side == 1 else ins['knn2']
        fkv_d = ins['fkv1'] if side == 1 else ins['fkv2']

        fkv = inp.tile([C + 3, N], F32, tag='fkv', name='fkv')
        nc.sync.dma_start(fkv[:], fkv_d[bi])

        # khat tile rows: 0-63 = normalized knn; 64-66 = pc^2; 67-69 = pc.
        # Rows 64-69 form the 6-row euclid key contract at base partition 64
        # (engine writes must start 32-aligned; matmul bases must be 0/32/64;
        # DMA writes are exempt).
        khat = keyp.tile([70, N], F32, tag='khat', name='khat')
        nc.scalar.activation(khat[C:C + 3, :], fkv[C:C + 3, :], AF.Square)
        nc.sync.dma_start(khat[C + 3:C + 6, :], fkv[C:C + 3, :])
        for j in range(N // CH):
            sl = slice(j * CH, (j + 1) * CH)
            knc = prep.tile([C, CH], F32, tag='knc', name='knc')
            nc.sync.dma_start(knc[:], knn_d[bi][:, sl])
            ksq = prep.tile([C, CH], F32, tag='scrA', name='ksq')
            nc.scalar.activation(ksq[:], knc[:], AF.Square)
            ssb = prep.tile([C, CH], F32, tag='scrB', name='ssb')
            nc.gpsimd.partition_all_reduce(ssb[:], ksq[:], channels=C,
                                           reduce_op=ROP.add)
            nc.scalar.activation(ssb[:], ssb[:], AF.Sqrt, bias=eps64[:])
            rinv = prep.tile([C, CH], F32, tag='scrA', name='rinv')
            nc.vector.reciprocal(rinv[:], ssb[:])
            nc.gpsimd.tensor_tensor(khat[0:C, sl], knc[:], rinv[:],
                                    op=ALU.mult)



        # akv2 = [W22;Wpos] @ [feat;pc] + b22, replicated to 128 partitions
        fkv16 = f16p.tile([C + 3, N], F16, tag='fkv16', name='fkv16')
        nc.gpsimd.tensor_copy(fkv16[:], fkv[0:C + 3, :])
        akv2 = keyp.tile([128, N], F32, tag='akv2', name='akv2')
        for kb in range(NB):
            sl = slice(kb * 512, (kb + 1) * 512)
            ps = p64_ps.tile([C, 512], F32, tag='p64', name='akv_ps')
            nc.tensor.matmul(ps[:], lhsT=w22pT[:], rhs=fkv16[:, sl],
                             start=True, stop=True)
            nc.scalar.activation(akv2[0:C, sl], ps[:], AF.Identity,
                                 bias=b22[:])
        nc.sync.dma_start(akv2[C:128, :], akv2[0:C, :])
        return dict(fkv=fkv, khat=khat, akv2=akv2)

    # ---------- per-combo query prep ----------
    def query_prep(sd_q):
        fkv = sd_q['fkv']
        # cq = W11@feat_q - Wpos@pc_q + (b11 + bpos)
        cq = qp.tile([C, Q], F32, tag='cq', name='cq')
        ps = p64_ps.tile([C, 512], F32, tag='p64', name='cq_ps')
        nc.tensor.matmul(ps[:, :Q], lhsT=w11pnT[:], rhs=fkv[0:C + 3, 0:Q],
                         start=True, stop=True)
        nc.scalar.activation(cq[:], ps[:, :Q], AF.Identity, bias=bqc[:])
        # augq rows 64-69: [-0.5, -0.5, -0.5, qx, qy, qz] — contracts with
        # khat_k rows 64-69 = [x^2, y^2, z^2, x, y, z] to give the euclid
        # ranking score  q.k - 0.5|k|^2  (monotone in -distance).
        augq = qp.tile([70, Q], F32, tag='augq', name='augq')
        nc.scalar.activation(augq[C:C + 3, :], fkv[C:C + 3, 0:Q],
                             AF.Copy, scale=0.0, bias=-0.5)
        nc.sync.dma_start(augq[C + 3:C + 6, :], fkv[C:C + 3, 0:Q])
        return dict(cq=cq, augq=augq)

    # ---------- tile stages ----------
    def tile_scores(cb):
        sd_q, sd_k, t = cb['q'], cb['k'], cb['t']
        tsl = slice(t * 128, (t + 1) * 128)
        khat_q, khat_k = sd_q['khat'], sd_k['khat']
        augq = cb['qd']['augq']

        sc_cos = scp.tile([128, N], F32, tag='sc', name='sc_cos')
        for cb2 in range(N // 1024):
            ps = sc_ps.tile([128, 1024], F32, tag='sc_ps', name='sc_ps')
            for h in range(2):
                sl = slice(cb2 * 1024 + h * 512, cb2 * 1024 + (h + 1) * 512)
                nc.tensor.matmul(ps[:, h * 512:(h + 1) * 512],
                                 lhsT=khat_q[0:C, tsl], rhs=khat_k[0:C, sl],
                                 start=True, stop=True)
            nc.scalar.activation(sc_cos[:, cb2 * 1024:(cb2 + 1) * 1024],
                                 ps[:], AF.Copy)
        sc_euc = scp.tile([128, N], F32, tag='sc', name='sc_euc')
        for cb2 in range(N // 1024):
            ps = sc_ps.tile([128, 1024], F32, tag='sc_ps', name='sc_ps')
            for h in range(2):
                sl = slice(cb2 * 1024 + h * 512, cb2 * 1024 + (h + 1) * 512)
                nc.tensor.matmul(ps[:, h * 512:(h + 1) * 512],
                                 lhsT=augq[C:C + 6, tsl],
                                 rhs=khat_k[C:C + 6, sl],
                                 start=True, stop=True)
            nc.scalar.activation(sc_euc[:, cb2 * 1024:(cb2 + 1) * 1024],
                                 ps[:], AF.Copy)
        cb['sc_cos'], cb['sc_euc'] = sc_cos, sc_euc
        if taps is not None and cb['ci'] == 0 and cb['t'] == 0:
            nc.sync.dma_start(taps['dbg_khat'], sd_k['khat'][:])
            nc.sync.dma_start(taps['dbg_akv2'], sd_k['akv2'][:])
            nc.sync.dma_start(taps['dbg_cq'], cb['qd']['cq'][:])
            nc.sync.dma_start(taps['dbg_augq'], augq[:])
            nc.sync.dma_start(taps['dbg_sccos'], sc_cos[:])
            nc.sync.dma_start(taps['dbg_sceuc'], sc_euc[:])

    def tile_topk(cb):
        vals = idxp.tile([128, 16], F32, tag='vals', name='vals')
        idxu = idxp.tile([128, 16], U16, tag='idxu', name='idxu')
        nc.vector.max(vals[:, 0:8], cb['sc_cos'][:])
        nc.vector.max_index(idxu[:, 0:8], vals[:, 0:8], cb['sc_cos'][:])
        nc.vector.max(vals[:, 8:16], cb['sc_euc'][:])
        nc.vector.max_index(idxu[:, 8:16], vals[:, 8:16], cb['sc_euc'][:])
        cb['idxu'] = idxu

    def tile_post(cb):
        sd_q, sd_k, t = cb['q'], cb['k'], cb['t']
        tsl = slice(t * 128, (t + 1) * 128)
        cq = cb['qd']['cq']
        akv2 = sd_k['akv2']

        # ---- index transpose to gather layout ----
        idxf = idxp.tile([128, 16], F32, tag='idxf', name='idxf')
        nc.vector.tensor_copy(idxf[:], cb['idxu'][:])
        pst = tp_ps.tile([16, 128], F32, tag='tp', name='pst')
        nc.tensor.matmul(pst[:], lhsT=idxf[:], rhs=id128[:],
                         start=True, stop=True)
        idxrow = idxp.tile([16, 128], I16, tag='idxrow', name='idxrow')
        nc.scalar.activation(idxrow[:], pst[:], AF.Copy)
        idxT = idxp.tile([128, 64], I16, tag='idxT', name='idxT')
        for h in range(2):
            b = h * 64
            nc.sync.dma_start(idxT[b:b + 16, :],
                              idxrow[:, h * 64:(h + 1) * 64])
            nc.sync.dma_start(idxT[b + 16:b + 32, :], idxT[b:b + 16, :])
            nc.sync.dma_start(idxT[b + 32:b + 64, :], idxT[b:b + 32, :])

        # ---- gather (stacked 128-partition layout) ----
        ag = mlpp.tile([128, 1024], F32, tag='ag', name='ag', bufs=2)
        nc.gpsimd.ap_gather(ag[:], akv2[:], idxT[:], channels=128,
                            num_elems=N, d=1, num_idxs=1024)
        if taps is not None and cb['ci'] == 0 and t == 0:
            nc.sync.dma_start(taps['dbg_idxf'], idxf[:])
            nc.sync.dma_start(taps['dbg_idxT'], idxT[:])
            nc.sync.dma_start(taps['dbg_ag'], ag[:])

        # ---- layer 0: + per-query cq, leaky ----
        # cross-partition copies: scalar engine only (gpsimd is per-core)
        cqs = mlpp.tile([128, 64], F32, tag='cqs', name='cqs', bufs=2)
        for h in range(2):
            nc.scalar.activation(
                cqs[h * 64:(h + 1) * 64, :],
                cq[:, t * 128 + h * 64:t * 128 + (h + 1) * 64], AF.Copy)
        nc.vector.tensor_tensor(
            ag[:].rearrange('c (q k) -> c q k', k=NS),
            ag[:].rearrange('c (q k) -> c q k', k=NS),
            cqs[:].to_broadcast([128, 64, NS]), op=ALU.add)
        n1 = mlpp.tile([128, 1024], F16, tag='n1', name='n1')
        nc.scalar.activation(n1[:], ag[:], AF.Prelu, alpha=LEAKY)
        if taps is not None and cb['ci'] == 0 and t == 0:
            nc.sync.dma_start(taps['dbg_n1'], n1[:])

        # ---- layers 1, 2 (fp16, stacked weights) ----
        cur = n1
        for li, (w, bias) in enumerate(((wm1stk, bm1s), (wm2stk, bm2s))):
            ps = mlp_ps.tile([128, 1024], F32, tag='mlp', name='mlp_ps')
            for h in range(2):
                hs = slice(h * 512, (h + 1) * 512)
                nc.tensor.matmul(ps[:, hs], lhsT=w[:], rhs=cur[:, hs],
                                 start=True, stop=True)
            if li == 0:
                h1 = mlpp.tile([128, 1024], F16, tag='h1', name='h1')
                nc.scalar.activation(h1[:], ps[:], AF.Prelu, bias=bias[:],
                                     alpha=LEAKY)
                cur = h1
            else:
                h2 = mlpp.tile([128, 1024], F16, tag='h2', name='h2')
                nc.scalar.activation(h2[:], ps[:], AF.Prelu, bias=bias[:],
                                     alpha=LEAKY)
                if taps is not None and cb['ci'] == 0 and t == 0:
                    nc.sync.dma_start(taps['dbg_h2'], h2[:])
                cur = h2

        # ---- maxpool over NS neighbors (DVE pair tree, fp16 2x mode) ----
        width = NS
        while width > 1:
            w2 = width // 2
            if w2 == 1:
                nxt = outp.tile([128, 64], F32, tag='ot', name='ot')
            else:
                nxt = mlpp.tile([128, 64 * w2], F16, tag=f'mp{w2}',
                                name=f'mp{w2}')
            v = cur[:].rearrange('c (q w two) -> c q w two', two=2, w=w2)
            nc.vector.tensor_tensor(
                nxt[:].rearrange('c (q w) -> c q w', w=w2),
                v[:, :, :, 0], v[:, :, :, 1], op=ALU.max)
            cur = nxt
            width = w2

        # ---- out DMA: [128, 64] (2q-half x 64ch, 64q) -> [64, 128] ----
        outap = cb['outap'][cb['bi']]
        dst = bass.AP(outap.tensor, outap.offset + t * 128,
                      [[64, 2], [Q, 64], [1, 64]])
        nc.sync.dma_start(dst, cur[:])

    # ---------- main loop (2-stage software pipeline) ----------
    pending = None
    ci = 0
    for bi in range(B):
        sd = {1: key_prep(bi, 1), 2: key_prep(bi, 2)}
        for outap, qs, ks in ((out1, 1, 2), (out2, 2, 1)):
            qd = query_prep(sd[qs])
            for t in range(NT):
                cb = dict(q=sd[qs], k=sd[ks], qd=qd, t=t, bi=bi, outap=outap,
                          ci=ci)
                tile_scores(cb)
                if pending is not None:
                    tile_post(pending)
                tile_topk(cb)
                pending = cb
            ci += 1
    tile_post(pending)
    ctx.close()


# ======================= host side =======================

_CACHED = {}


def _get_nc():
    if 'nc' not in _CACHED:
        _CACHED['nc'] = build_nc()
    return _CACHED['nc']


def make_in_maps(pc1, pc2, feat1, feat2, knn1, knn2,
                 W_t11, b_t11, W_t22, b_t22, W_pos, b_pos,
                 W_m1, b_m1, W_m2, b_m2):
    f32, f16 = np.float32, np.float16
    W_t11 = np.asarray(W_t11, f32); W_t22 = np.asarray(W_t22, f32)
    W_pos = np.asarray(W_pos, f32)
    W_m1 = np.asarray(W_m1, f32); W_m2 = np.asarray(W_m2, f32)

    w22pT = np.vstack([W_t22.T, W_pos.T]).astype(f16)            # [67, 64]
    w11pnT = np.vstack([W_t11.T, -W_pos.T]).astype(f32)          # [67, 64]
    z = np.zeros((C, C), f32)
    wm1stk = np.block([[W_m1.T, z], [z, W_m1.T]]).astype(f16)    # [128,128]
    wm2stk = np.block([[W_m2.T, z], [z, W_m2.T]]).astype(f16)
    b22 = np.asarray(b_t22, f32).reshape(C, 1)
    bqc = (np.asarray(b_t11, f32) + np.asarray(b_pos, f32)).reshape(C, 1)
    bm1s = np.tile(np.asarray(b_m1, f32).reshape(C, 1), (2, 1))
    bm2s = np.tile(np.asarray(b_m2, f32).reshape(C, 1), (2, 1))

    fkv1 = np.concatenate([np.asarray(feat1, f32), np.asarray(pc1, f32)], 1)
    fkv2 = np.concatenate([np.asarray(feat2, f32), np.asarray(pc2, f32)], 1)
    knn1 = np.asarray(knn1, f32)
    knn2 = np.asarray(knn2, f32)

    base = {
        'w22pT': w22pT, 'w11pnT': w11pnT,
        'wm1stk': wm1stk, 'wm2stk': wm2stk,
        'b22': b22, 'bqc': bqc, 'bm1s': bm1s, 'bm2s': bm2s,
        'id128': np.eye(128, dtype=f32),
    }
    in_maps = []
    for c in range(NCORES):
        m = dict(base)
        r = -c * Q
        m['knn1'] = np.ascontiguousarray(np.roll(knn1, r, axis=2))
        m['knn2'] = np.ascontiguousarray(np.roll(knn2, r, axis=2))
        m['fkv1'] = np.ascontiguousarray(np.roll(fkv1, r, axis=2))
        m['fkv2'] = np.ascontiguousarray(np.roll(fkv2, r, axis=2))
        in_maps.append(m)
    return in_maps


def kernel(pc1, pc2, feat1, feat2, knn1, knn2,
           W_t11, b_t11, W_t22, b_t22, W_pos, b_pos,
           W_m1, b_m1, W_m2, b_m2):
    from concourse.bass_utils import run_bass_kernel_spmd
    nc = _get_nc()
    in_maps = make_in_maps(pc1, pc2, feat1, feat2, knn1, knn2,
                           W_t11, b_t11, W_t22, b_t22, W_pos, b_pos,
                           W_m1, b_m1, W_m2, b_m2)
    res = run_bass_kernel_spmd(nc, in_maps, core_ids=list(range(NCORES)))
    out1 = np.concatenate([res.results[c]['out1'] for c in range(NCORES)],
                          axis=2)
    out2 = np.concatenate([res.results[c]['out2'] for c in range(NCORES)],
                          axis=2)
    return out1, out2


# revision 44
# speedup vs baseline: 56.0167x; 1.1355x over previous
"""Trainium2 Bass kernel for nn_BidirectionalLayerFeatCosine (retrieval_knn).

Strategy: shard the 4096 query points across 8 NeuronCores (512 each); keys
are replicated.  Host rolls the key axis per core so each core's query block
is always columns 0:512 (SPMD-clean static slices).

Per core, per batch, per side: ONE wide DMA loads [feat; _; pc; pc-dup]
(fkv tile, rows 0-63 / 64-66(pc^2 target) / 67-69 / 96-98); knn is loaded in
chunks and normalized exactly in fp32 via gpsimd partition_all_reduce + ACT
sqrt + DVE reciprocal + gpsimd multiply -> khat (query side is a slice).
akv = W22@feat + Wpos@pc + b22 via one fp16 70-row matmul per chunk,
replicated to 128 partitions for the stacked gather.  Scores (cos + euclid)
stay exact fp32 on the PE; top-8 via DVE max8/find_index8; ap_gather pulls
neighbors (and a second static-index gather expands cq); the fp16 MLP uses
block-diagonal stacked weights; maxpool pair-tree; per-combo output
accumulation with one contiguous DMA emitted a combo late.
"""
import sys

for _p in ('/opt/trn_rl_repo',):
    if _p not in sys.path:
        sys.path.insert(0, _p)

import numpy as np
import concourse.bass as bass
import concourse.tile as tile
from concourse import bacc, mybir, bass_isa

F32 = mybir.dt.float32
F16 = mybir.dt.float16
I16 = mybir.dt.int16
U16 = mybir.dt.uint16
AF = mybir.ActivationFunctionType
ALU = mybir.AluOpType
ROP = bass_isa.ReduceOp

B, N, C, NS = 2, 4096, 64, 16
NCORES = 8
Q = N // NCORES           # queries per core per combo (512)
NT = Q // 128             # query tiles per combo (4)
LEAKY = 0.1
EPS = 1e-8
FKR = 99                  # fkv tile rows


def build_nc(debug_taps=False):
    nc = bacc.Bacc("TRN2", num_devices=NCORES, debug=False)

    def din(name, shape, dt=F32):
        return nc.dram_tensor(name, list(shape), dt, kind="ExternalInput").ap()

    ins = {
        'knn1': din('knn1', (B, C, N)),
        'knn2': din('knn2', (B, C, N)),
        'fkv1': din('fkv1', (B, FKR, N)),
        'fkv2': din('fkv2', (B, FKR, N)),
        'w22pT': din('w22pT', (70, C), F16),
        'w11pnT': din('w11pnT', (70, C)),
        'wm1stk': din('wm1stk', (128, 128), F16),
        'wm2stk': din('wm2stk', (128, 128), F16),
        'b22': din('b22', (C, 1)),
        'bqc': din('bqc', (C, 1)),
        'bm1s': din('bm1s', (128, 1)),
        'bm2s': din('bm2s', (128, 1)),
        'id128': din('id128', (128, 128)),
        'cqidx': din('cqidx', (128, 64 * NT), I16),
    }
    # out[b, h, c, t*64+q] = feat_new[b, c, 512*core + t*128 + 64*h + q]
    out1 = nc.dram_tensor('out1', [B, 2, C, 64 * NT], F32,
                          kind="ExternalOutput").ap()
    out2 = nc.dram_tensor('out2', [B, 2, C, 64 * NT], F32,
                          kind="ExternalOutput").ap()
    taps = None
    if debug_taps:
        taps = {nm: nc.dram_tensor(nm, list(sh), dt, kind="ExternalOutput").ap()
                for nm, sh, dt in [
                    ('dbg_khat', (C, N), F32),
                    ('dbg_fkv', (FKR, N), F32),
                    ('dbg_akv2', (128, N), F32),
                    ('dbg_cq2', (128, Q), F32),
                    ('dbg_augq', (70, Q), F32),
                    ('dbg_sccos', (128, N), F32),
                    ('dbg_sceuc', (128, N), F32),
                    ('dbg_idxf', (128, 16), F32),
                    ('dbg_ag', (128, 1024), F32),
                    ('dbg_cqs', (128, 1024), F32),
                    ('dbg_h2', (128, 1024), F16),
                ]}

    with tile.TileContext(nc) as tc:
        _body(tc, ins, out1, out2, taps)
    nc.compile()
    return nc


def _body(tc, ins, out1, out2, taps=None):
    nc = tc.nc
    from contextlib import ExitStack
    ctx = ExitStack()

    pool = lambda name, bufs, space='SBUF': ctx.enter_context(
        tc.tile_pool(name=name, bufs=bufs, space=space))

    consts = pool('consts', 1)
    inp = pool('inputs', 3)        # fkv tiles (rotate across sides/batches)
    prep = pool('prep', 2)         # chunked scratch for normalization
    keyp = pool('keyprep', 2)      # khat / akv2 (both sides live)
    f16p = pool('f16', 1)
    qp = pool('qside', 2)          # cq2 / augq per combo
    scp = pool('scores', 2)        # [128, 4096] score rows
    idxp = pool('idx', 2)          # vals/idx tiles per tile
    mlpp = pool('mlp', 1)
    outp = pool('out', 2)

    sc_ps = ctx.enter_context(tc.tile_pool(name='sc_ps', bufs=2, space='PSUM'))
    mlp_ps = ctx.enter_context(tc.tile_pool(name='mlp_ps', bufs=1, space='PSUM'))
    p64_ps = ctx.enter_context(tc.tile_pool(name='p64_ps', bufs=1, space='PSUM'))
    tp_ps = ctx.enter_context(tc.tile_pool(name='tp_ps', bufs=1, space='PSUM'))

    # ---- constants ----
    def cload(name, shape, dt=F32):
        t = consts.tile(list(shape), dt, tag=name, name=name)
        nc.sync.dma_start(t[:], ins[name])
        return t

    w22pT = cload('w22pT', (70, C), F16)
    w11pnT = cload('w11pnT', (70, C))
    wm1stk = cload('wm1stk', (128, 128), F16)
    wm2stk = cload('wm2stk', (128, 128), F16)
    b22 = cload('b22', (C, 1))
    bqc = cload('bqc', (C, 1))
    bm1s = cload('bm1s', (128, 1))
    bm2s = cload('bm2s', (128, 1))
    id128 = cload('id128', (128, 128))
    cqidx = cload('cqidx', (128, 64 * NT), I16)
    eps64 = consts.tile([C, 1], F32, tag='eps64', name='eps64')
    nc.vector.memset(eps64[:], EPS)

    # ---------- per (batch, side) key prep ----------
    CH = 512                       # normalization chunk width

    def key_prep(bi, side):
        knn_d = ins['knn1'] if side == 1 else ins['knn2']
        fkv_d = ins['fkv1'] if side == 1 else ins['fkv2']

        # fkv rows: 0-63 feat, 64-66 pc^2 (computed), 67-69 pc, 96-98 pc.
        fkv = inp.tile([FKR, N], F32, tag='fkv', name='fkv')
        nc.sync.dma_start(fkv[:], fkv_d[bi])
        nc.scalar.activation(fkv[C:C + 3, :], fkv[96:99, :], AF.Square)

        # khat = knn / sqrt(colsum(knn^2) + eps), chunked (exact fp32)
        khat = keyp.tile([C, N], F32, tag='khat', name='khat')
        for j in range(N // CH):
            sl = slice(j * CH, (j + 1) * CH)
            knc = prep.tile([C, CH], F32, tag='knc', name='knc')
            nc.sync.dma_start(knc[:], knn_d[bi][:, sl])
            ksq = prep.tile([C, CH], F32, tag='scrA', name='ksq')
            nc.gpsimd.tensor_tensor(ksq[:], knc[:], knc[:], op=ALU.mult)
            ssb = prep.tile([C, CH], F32, tag='scrB', name='ssb')
            nc.gpsimd.partition_all_reduce(ssb[:], ksq[:], channels=C,
                                           reduce_op=ROP.add)
            nc.scalar.activation(ssb[:], ssb[:], AF.Sqrt, bias=eps64[:])
            rinv = prep.tile([C, CH], F32, tag='scrA', name='rinv')
            nc.vector.reciprocal(rinv[:], ssb[:])
            nc.gpsimd.tensor_tensor(khat[:, sl], knc[:], rinv[:],
                                    op=ALU.mult)

        # akv2 = [W22; 0; Wpos] @ fkv[0:70] + b22, replicated to 128 parts
        fkv16 = f16p.tile([70, N], F16, tag='fkv16', name='fkv16')
        nc.gpsimd.tensor_copy(fkv16[:], fkv[0:70, :])
        akv2 = keyp.tile([128, N], F32, tag='akv2', name='akv2')
        for kb in range(N // 512):
            sl = slice(kb * 512, (kb + 1) * 512)
            ps = p64_ps.tile([C, 512], F32, tag='p64', name='akv_ps')
            nc.tensor.matmul(ps[:], lhsT=w22pT[:], rhs=fkv16[:, sl],
                             start=True, stop=True)
            nc.scalar.activation(akv2[0:C, sl], ps[:], AF.Identity,
                                 bias=b22[:])
        nc.sync.dma_start(akv2[C:128, :], akv2[0:C, :])
        return dict(fkv=fkv, khat=khat, akv2=akv2)

    # ---------- per-combo query prep ----------
    def query_prep(sd_q):
        fkv = sd_q['fkv']
        # cq = W11@feat_q - Wpos@pc_q + (b11 + bpos), stacked to 128 rows
        cq2 = qp.tile([128, Q], F32, tag='cq2', name='cq2')
        ps = p64_ps.tile([C, 512], F32, tag='p64', name='cq_ps')
        nc.tensor.matmul(ps[:, :Q], lhsT=w11pnT[:], rhs=fkv[0:70, 0:Q],
                         start=True, stop=True)
        nc.scalar.activation(cq2[0:C, :], ps[:, :Q], AF.Identity,
                             bias=bqc[:])
        nc.scalar.activation(cq2[C:128, :], ps[:, :Q], AF.Identity,
                             bias=bqc[:])
        # augq rows 64-69: [-0.5 x3, qx, qy, qz] — contracts with fkv rows
        # 64-69 = [x^2, y^2, z^2, x, y, z]: score = q.k - 0.5|k|^2.
        augq = qp.tile([70, Q], F32, tag='augq', name='augq')
        nc.scalar.activation(augq[C:C + 3, :], fkv[C:C + 3, 0:Q],
                             AF.Copy, scale=0.0, bias=-0.5)
        nc.sync.dma_start(augq[C + 3:C + 6, :], fkv[C + 3:C + 6, 0:Q])
        return dict(cq2=cq2, augq=augq)

    # ---------- tile stages ----------
    def tile_scores(cb):
        sd_q, sd_k, t = cb['q'], cb['k'], cb['t']
        tsl = slice(t * 128, (t + 1) * 128)
        khat_q, khat_k = sd_q['khat'], sd_k['khat']
        fkv_k = sd_k['fkv']
        augq = cb['qd']['augq']

        sc_cos = scp.tile([128, N], F32, tag='sc', name='sc_cos')
        for j in range(N // 1024):
            ps = sc_ps.tile([128, 1024], F32, tag='sc_ps', name='sc_ps')
            for h in range(2):
                sl = slice(j * 1024 + h * 512, j * 1024 + (h + 1) * 512)
                nc.tensor.matmul(ps[:, h * 512:(h + 1) * 512],
                                 lhsT=khat_q[:, tsl], rhs=khat_k[:, sl],
                                 start=True, stop=True)
            nc.scalar.activation(sc_cos[:, j * 1024:(j + 1) * 1024],
                                 ps[:], AF.Copy)
        sc_euc = scp.tile([128, N], F32, tag='sc', name='sc_euc')
        for j in range(N // 1024):
            ps = sc_ps.tile([128, 1024], F32, tag='sc_ps', name='sc_ps')
            for h in range(2):
                sl = slice(j * 1024 + h * 512, j * 1024 + (h + 1) * 512)
                nc.tensor.matmul(ps[:, h * 512:(h + 1) * 512],
                                 lhsT=augq[C:C + 6, tsl],
                                 rhs=fkv_k[C:C + 6, sl],
                                 start=True, stop=True)
            nc.scalar.activation(sc_euc[:, j * 1024:(j + 1) * 1024],
                                 ps[:], AF.Copy)
        cb['sc_cos'], cb['sc_euc'] = sc_cos, sc_euc
        if taps is not None and cb['ci'] == 0 and t == 0:
            nc.sync.dma_start(taps['dbg_khat'], sd_k['khat'][:])
            nc.sync.dma_start(taps['dbg_fkv'], fkv_k[:])
            nc.sync.dma_start(taps['dbg_akv2'], sd_k['akv2'][:])
            nc.sync.dma_start(taps['dbg_cq2'], cb['qd']['cq2'][:])
            nc.sync.dma_start(taps['dbg_augq'], augq[:])
            nc.sync.dma_start(taps['dbg_sccos'], sc_cos[:])
            nc.sync.dma_start(taps['dbg_sceuc'], sc_euc[:])

    def tile_topk(cb):
        vals = idxp.tile([128, 16], F32, tag='vals', name='vals')
        idxu = idxp.tile([128, 16], U16, tag='idxu', name='idxu')
        nc.vector.max(vals[:, 0:8], cb['sc_cos'][:])
        nc.vector.max_index(idxu[:, 0:8], vals[:, 0:8], cb['sc_cos'][:])
        nc.vector.max(vals[:, 8:16], cb['sc_euc'][:])
        nc.vector.max_index(idxu[:, 8:16], vals[:, 8:16], cb['sc_euc'][:])
        cb['idxu'] = idxu

    def tile_post(cb):
        sd_k, t = cb['k'], cb['t']
        cq2 = cb['qd']['cq2']
        akv2 = sd_k['akv2']

        # ---- index transpose to gather layout ----
        idxf = idxp.tile([128, 16], F32, tag='idxf', name='idxf')
        nc.vector.tensor_copy(idxf[:], cb['idxu'][:])
        pst = tp_ps.tile([16, 128], F32, tag='tp', name='pst')
        nc.tensor.matmul(pst[:], lhsT=idxf[:], rhs=id128[:],
                         start=True, stop=True)
        idxrow = idxp.tile([16, 128], I16, tag='idxrow', name='idxrow')
        nc.scalar.activation(idxrow[:], pst[:], AF.Copy)
        idxT = idxp.tile([128, 64], I16, tag='idxT', name='idxT')
        for h in range(2):
            b = h * 64
            nc.sync.dma_start(idxT[b:b + 16, :],
                              idxrow[:, h * 64:(h + 1) * 64])
            nc.sync.dma_start(idxT[b + 16:b + 32, :], idxT[b:b + 16, :])
            nc.sync.dma_start(idxT[b + 32:b + 64, :], idxT[b:b + 32, :])

        # ---- gathers: neighbors + per-query cq expansion ----
        ag = mlpp.tile([128, 1024], F32, tag='ag', name='ag', bufs=2)
        nc.gpsimd.ap_gather(ag[:], akv2[:], idxT[:], channels=128,
                            num_elems=N, d=1, num_idxs=1024)
        cqs = mlpp.tile([128, 1024], F32, tag='cqs', name='cqs')
        nc.gpsimd.ap_gather(cqs[:], cq2[:], cqidx[:, t * 64:(t + 1) * 64],
                            channels=128, num_elems=Q, d=1, num_idxs=1024)
        if taps is not None and cb['ci'] == 0 and t == 0:
            nc.sync.dma_start(taps['dbg_idxf'], idxf[:])
            nc.sync.dma_start(taps['dbg_ag'], ag[:])
            nc.sync.dma_start(taps['dbg_cqs'], cqs[:])

        # ---- layer 0: add cq, leaky (-> fp16) ----
        nc.gpsimd.tensor_tensor(ag[:], ag[:], cqs[:], op=ALU.add)
        n1 = mlpp.tile([128, 1024], F16, tag='n1', name='n1')
        nc.scalar.activation(n1[:], ag[:], AF.Prelu, alpha=LEAKY)

        # ---- layers 1, 2 (fp16, stacked weights) ----
        cur = n1
        for li, (w, bias) in enumerate(((wm1stk, bm1s), (wm2stk, bm2s))):
            ps = mlp_ps.tile([128, 1024], F32, tag='mlp', name='mlp_ps')
            for h in range(2):
                hs = slice(h * 512, (h + 1) * 512)
                nc.tensor.matmul(ps[:, hs], lhsT=w[:], rhs=cur[:, hs],
                                 start=True, stop=True)
            ht = mlpp.tile([128, 1024], F16, tag=f'h{li + 1}',
                           name=f'h{li + 1}')
            nc.scalar.activation(ht[:], ps[:], AF.Prelu, bias=bias[:],
                                 alpha=LEAKY)
            cur = ht
        if taps is not None and cb['ci'] == 0 and t == 0:
            nc.sync.dma_start(taps['dbg_h2'], cur[:])

        # ---- maxpool over NS neighbors (pair tree, fp16 2x mode) ----
        width = NS
        while width > 1:
            w2 = width // 2
            if w2 == 1:
                nxt = cb['outc']
                dst = nxt[:, t * 64:(t + 1) * 64]
            else:
                nxt = mlpp.tile([128, 64 * w2], F16, tag=f'mp{w2}',
                                name=f'mp{w2}')
                dst = nxt[:]
            v = cur[:].rearrange('c (q w two) -> c q w two', two=2, w=w2)
            nc.vector.tensor_tensor(
                dst.rearrange('c (q w) -> c q w', w=w2),
                v[:, :, :, 0], v[:, :, :, 1], op=ALU.max)
            cur = nxt
            width = w2

    def emit_out(cb):
        # one contiguous DMA per combo: [128, 256] -> out[b, h, c, :]
        outap = cb['outap']
        base = outap.offset + cb['bi'] * 2 * C * 64 * NT
        dst = bass.AP(outap.tensor, base,
                      [[C * 64 * NT, 2], [64 * NT, C], [1, 64 * NT]])
        nc.sync.dma_start(dst, cb['outc'][:])

    # ---------- main loop (software pipelined) ----------
    pending = None
    pending_out = None
    ci = 0
    for bi in range(B):
        sd = {1: key_prep(bi, 1), 2: key_prep(bi, 2)}
        for outap, qs, ks in ((out1, 1, 2), (out2, 2, 1)):
            qd = query_prep(sd[qs])
            outc = outp.tile([128, 64 * NT], F32, tag='outc', name='outc')
            for t in range(NT):
                cb = dict(q=sd[qs], k=sd[ks], qd=qd, t=t, bi=bi,
                          outap=outap, outc=outc, ci=ci)
                tile_scores(cb)
                if pending is not None:
                    tile_post(pending)
                if pending_out is not None and ci > pending_out[0] \
                        and t == 1:
                    emit_out(pending_out[1])
                    pending_out = None
                tile_topk(cb)
                pending = cb
            pending_out = (ci, cb)
            ci += 1
    tile_post(pending)
    emit_out(pending_out[1])
    ctx.close()


# ======================= host side =======================

_CACHED = {}


def _get_nc():
    if 'nc' not in _CACHED:
        _CACHED['nc'] = build_nc()
    return _CACHED['nc']


def make_in_maps(pc1, pc2, feat1, feat2, knn1, knn2,
                 W_t11, b_t11, W_t22, b_t22, W_pos, b_pos,
                 W_m1, b_m1, W_m2, b_m2):
    f32, f16 = np.float32, np.float16
    W_t11 = np.asarray(W_t11, f32); W_t22 = np.asarray(W_t22, f32)
    W_pos = np.asarray(W_pos, f32)
    W_m1 = np.asarray(W_m1, f32); W_m2 = np.asarray(W_m2, f32)

    z3 = np.zeros((3, C), f32)
    w22pT = np.vstack([W_t22.T, z3, W_pos.T]).astype(f16)        # [70, 64]
    w11pnT = np.vstack([W_t11.T, z3, -W_pos.T]).astype(f32)      # [70, 64]
    z = np.zeros((C, C), f32)
    wm1stk = np.block([[W_m1.T, z], [z, W_m1.T]]).astype(f16)    # [128,128]
    wm2stk = np.block([[W_m2.T, z], [z, W_m2.T]]).astype(f16)
    b22 = np.asarray(b_t22, f32).reshape(C, 1)
    bqc = (np.asarray(b_t11, f32) + np.asarray(b_pos, f32)).reshape(C, 1)
    bm1s = np.tile(np.asarray(b_m1, f32).reshape(C, 1), (2, 1))
    bm2s = np.tile(np.asarray(b_m2, f32).reshape(C, 1), (2, 1))

    # cqidx[16g+s, t*64+q] = t*128 + 64*(g>=4) + q: groups 0-3 expand the
    # first 64 queries of tile t, groups 4-7 the second 64.
    cqidx = np.zeros((128, 64 * NT), np.int16)
    for g in range(8):
        h = g // 4
        for t in range(NT):
            cqidx[16 * g:16 * (g + 1), t * 64:(t + 1) * 64] = \
                t * 128 + 64 * h + np.arange(64, dtype=np.int16)[None, :]

    def build_fkv(feat, pc):
        b, _, n = feat.shape
        fkv = np.zeros((b, FKR, n), f32)
        fkv[:, 0:C] = feat
        fkv[:, C + 3:C + 6] = pc
        fkv[:, 96:99] = pc
        return fkv

    fkv1 = build_fkv(np.asarray(feat1, f32), np.asarray(pc1, f32))
    fkv2 = build_fkv(np.asarray(feat2, f32), np.asarray(pc2, f32))
    knn1 = np.asarray(knn1, f32)
    knn2 = np.asarray(knn2, f32)

    base = {
        'w22pT': w22pT, 'w11pnT': w11pnT,
        'wm1stk': wm1stk, 'wm2stk': wm2stk,
        'b22': b22, 'bqc': bqc, 'bm1s': bm1s, 'bm2s': bm2s,
        'id128': np.eye(128, dtype=f32),
        'cqidx': cqidx,
    }
    in_maps = []
    for c in range(NCORES):
        m = dict(base)
        r = -c * Q
        m['knn1'] = np.ascontiguousarray(np.roll(knn1, r, axis=2))
        m['knn2'] = np.ascontiguousarray(np.roll(knn2, r, axis=2))
        m['fkv1'] = np.ascontiguousarray(np.roll(fkv1, r, axis=2))
        m['fkv2'] = np.ascontiguousarray(np.roll(fkv2, r, axis=2))
        in_maps.append(m)
    return in_maps


def _unstack_out(res, name):
    # per-core out [B, 2, C, 256] -> [B, C, 512] block, concat on queries
    blocks = []
    for c in range(NCORES):
        o = res.results[c][name]           # [B, 2, C, 256]
        o = o.reshape(B, 2, C, NT, 64)     # [b, h, c, t, q]
        o = o.transpose(0, 2, 3, 1, 4)     # [b, c, t, h, q]
        blocks.append(o.reshape(B, C, Q))
    return np.concatenate(blocks, axis=2)


def kernel(pc1, pc2, feat1, feat2, knn1, knn2,
           W_t11, b_t11, W_t22, b_t22, W_pos, b_pos,
           W_m1, b_m1, W_m2, b_m2):
    from concourse.bass_utils import run_bass_kernel_spmd
    nc = _get_nc()
    in_maps = make_in_maps(pc1, pc2, feat1, feat2, knn1, knn2,
                           W_t11, b_t11, W_t22, b_t22, W_pos, b_pos,
                           W_m1, b_m1, W_m2, b_m2)
    res = run_bass_kernel_spmd(nc, in_maps, core_ids=list(range(NCORES)))
    return _unstack_out(res, 'out1'), _unstack_out(res, 'out2')


# revision 50
# speedup vs baseline: 58.4439x; 1.0433x over previous
"""Trainium2 Bass kernel for nn_BidirectionalLayerFeatCosine (retrieval_knn).

Strategy: shard the 4096 query points across 8 NeuronCores (512 each); keys
are replicated.  Host rolls the key axis per core so each core's query block
is always columns 0:512 (SPMD-clean static slices).

Per core, per batch, per side: ONE wide DMA loads [feat; _; pc; pc-dup]
(fkv tile, rows 0-63 / 64-66(pc^2 target) / 67-69 / 96-98); knn is loaded in
chunks and normalized exactly in fp32 via gpsimd partition_all_reduce + ACT
sqrt + DVE reciprocal + gpsimd multiply -> khat (query side is a slice).
akv = W22@feat + Wpos@pc + b22 via one fp16 70-row matmul per chunk,
replicated to 128 partitions for the stacked gather.  Scores (cos + euclid)
stay exact fp32 on the PE; top-8 via DVE max8/find_index8; ap_gather pulls
neighbors (and a second static-index gather expands cq); the fp16 MLP uses
block-diagonal stacked weights; maxpool pair-tree; per-combo output
accumulation with one contiguous DMA emitted a combo late.
"""
import sys

for _p in ('/opt/trn_rl_repo',):
    if _p not in sys.path:
        sys.path.insert(0, _p)

import numpy as np
import concourse.bass as bass
import concourse.tile as tile
from concourse import bacc, mybir, bass_isa

F32 = mybir.dt.float32
F16 = mybir.dt.float16
I16 = mybir.dt.int16
U16 = mybir.dt.uint16
AF = mybir.ActivationFunctionType
ALU = mybir.AluOpType
ROP = bass_isa.ReduceOp

B, N, C, NS = 2, 4096, 64, 16
NCORES = 8
Q = N // NCORES           # queries per core per combo (512)
NT = Q // 128             # query tiles per combo (4)
LEAKY = 0.1
EPS = 1e-8
FKR = 99                  # fkv tile rows


def build_nc(debug_taps=False):
    nc = bacc.Bacc("TRN2", num_devices=NCORES, debug=False)

    def din(name, shape, dt=F32):
        return nc.dram_tensor(name, list(shape), dt, kind="ExternalInput").ap()

    ins = {
        'knn1': din('knn1', (B, C, N)),
        'knn2': din('knn2', (B, C, N)),
        'fkv1': din('fkv1', (B, FKR, N)),
        'fkv2': din('fkv2', (B, FKR, N)),
        'w22pT': din('w22pT', (70, C), F16),
        'w11pnT': din('w11pnT', (70, C)),
        'wm1stk': din('wm1stk', (128, 128), F16),
        'wm2stk': din('wm2stk', (128, 128), F16),
        'b22': din('b22', (C, 1)),
        'bqc': din('bqc', (C, 1)),
        'bm1s': din('bm1s', (128, 1)),
        'bm2s': din('bm2s', (128, 1)),
        'id128': din('id128', (128, 128)),
        'cqidx': din('cqidx', (128, 64 * NT), I16),
    }
    # out[b, h, c, t*64+q] = feat_new[b, c, 512*core + t*128 + 64*h + q]
    out1 = nc.dram_tensor('out1', [B, 2, C, 64 * NT], F32,
                          kind="ExternalOutput").ap()
    out2 = nc.dram_tensor('out2', [B, 2, C, 64 * NT], F32,
                          kind="ExternalOutput").ap()
    taps = None
    if debug_taps:
        taps = {nm: nc.dram_tensor(nm, list(sh), dt, kind="ExternalOutput").ap()
                for nm, sh, dt in [
                    ('dbg_khat', (C, N), F32),
                    ('dbg_fkv', (FKR, N), F32),
                    ('dbg_akv2', (128, N), F32),
                    ('dbg_cq2', (128, Q), F32),
                    ('dbg_augq', (70, Q), F32),
                    ('dbg_sccos', (128, N), F32),
                    ('dbg_sceuc', (128, N), F32),
                    ('dbg_idxf', (128, 16), F32),
                    ('dbg_ag', (128, 1024), F32),
                    ('dbg_cqs', (128, 1024), F32),
                    ('dbg_h2', (128, 1024), F16),
                ]}

    with tile.TileContext(nc) as tc:
        _body(tc, ins, out1, out2, taps)
    nc.compile()
    return nc


def _body(tc, ins, out1, out2, taps=None):
    nc = tc.nc
    from contextlib import ExitStack
    ctx = ExitStack()

    pool = lambda name, bufs, space='SBUF': ctx.enter_context(
        tc.tile_pool(name=name, bufs=bufs, space=space))

    consts = pool('consts', 1)
    inp = pool('inputs', 2)        # fkv tiles (rotate across sides/batches)
    prep = pool('prep', 2)         # chunked scratch for normalization
    keyp = pool('keyprep', 2)      # khat / akv2 (both sides live)
    f16p = pool('f16', 1)
    qp = pool('qside', 2)          # cq2 / augq per combo
    scp = pool('scores', 2)        # [128, 4096] score rows
    idxp = pool('idx', 2)          # vals/idx tiles per tile
    mlpp = pool('mlp', 1)
    outp = pool('out', 2)

    sc_ps = ctx.enter_context(tc.tile_pool(name='sc_ps', bufs=2, space='PSUM'))
    mlp_ps = ctx.enter_context(tc.tile_pool(name='mlp_ps', bufs=1, space='PSUM'))
    p64_ps = ctx.enter_context(tc.tile_pool(name='p64_ps', bufs=1, space='PSUM'))
    tp_ps = ctx.enter_context(tc.tile_pool(name='tp_ps', bufs=1, space='PSUM'))

    # ---- constants ----
    def cload(name, shape, dt=F32):
        t = consts.tile(list(shape), dt, tag=name, name=name)
        nc.sync.dma_start(t[:], ins[name])
        return t

    w22pT = cload('w22pT', (70, C), F16)
    w11pnT = cload('w11pnT', (70, C))
    wm1stk = cload('wm1stk', (128, 128), F16)
    wm2stk = cload('wm2stk', (128, 128), F16)
    b22 = cload('b22', (C, 1))
    bqc = cload('bqc', (C, 1))
    bm1s = cload('bm1s', (128, 1))
    bm2s = cload('bm2s', (128, 1))
    id128 = cload('id128', (128, 128))
    cqidx = cload('cqidx', (128, 64 * NT), I16)
    eps64 = consts.tile([C, 1], F32, tag='eps64', name='eps64')
    nc.vector.memset(eps64[:], EPS)

    # ---------- per (batch, side) key prep ----------
    CH = 512                       # normalization chunk width

    def key_prep(bi, side):
        knn_d = ins['knn1'] if side == 1 else ins['knn2']
        fkv_d = ins['fkv1'] if side == 1 else ins['fkv2']

        # fkv rows: 0-63 feat, 64-66 pc^2 (computed), 67-69 pc, 96-98 pc.
        fkv = inp.tile([FKR, N], F32, tag='fkv', name='fkv')
        nc.sync.dma_start(fkv[:], fkv_d[bi])
        nc.scalar.activation(fkv[C:C + 3, :], fkv[96:99, :], AF.Square)

        # khat = knn / sqrt(colsum(knn^2) + eps), exact fp32; one wide DMA
        # then chunked compute (no per-chunk DMA semaphore chains)
        knn = prep.tile([C, N], F32, tag='knn', name='knn', bufs=1)
        nc.sync.dma_start(knn[:], knn_d[bi])
        khat = keyp.tile([C, N], F32, tag='khat', name='khat')
        for j in range(N // CH):
            sl = slice(j * CH, (j + 1) * CH)
            ksq = prep.tile([C, CH], F32, tag='scrA', name='ksq')
            nc.gpsimd.tensor_tensor(ksq[:], knn[:, sl], knn[:, sl],
                                    op=ALU.mult)
            ssb = prep.tile([C, CH], F32, tag='scrB', name='ssb')
            nc.gpsimd.partition_all_reduce(ssb[:], ksq[:], channels=C,
                                           reduce_op=ROP.add)
            nc.scalar.activation(ssb[:], ssb[:], AF.Sqrt, bias=eps64[:])
            rinv = prep.tile([C, CH], F32, tag='scrA', name='rinv')
            nc.vector.reciprocal(rinv[:], ssb[:])
            nc.gpsimd.tensor_tensor(khat[:, sl], knn[:, sl], rinv[:],
                                    op=ALU.mult)

        # akv2 = [W22; 0; Wpos] @ fkv[0:70] + b22, replicated to 128 parts
        fkv16 = f16p.tile([70, N], F16, tag='fkv16', name='fkv16')
        nc.gpsimd.tensor_copy(fkv16[:], fkv[0:70, :])
        akv2 = keyp.tile([128, N], F32, tag='akv2', name='akv2')
        for kb in range(N // 512):
            sl = slice(kb * 512, (kb + 1) * 512)
            ps = p64_ps.tile([C, 512], F32, tag='p64', name='akv_ps')
            nc.tensor.matmul(ps[:], lhsT=w22pT[:], rhs=fkv16[:, sl],
                             start=True, stop=True)
            nc.scalar.activation(akv2[0:C, sl], ps[:], AF.Identity,
                                 bias=b22[:])
        nc.sync.dma_start(akv2[C:128, :], akv2[0:C, :])
        return dict(fkv=fkv, khat=khat, akv2=akv2)

    # ---------- per-combo query prep ----------
    def query_prep(sd_q):
        fkv = sd_q['fkv']
        # cq = W11@feat_q - Wpos@pc_q + (b11 + bpos), stacked to 128 rows
        cq2 = qp.tile([128, Q], F32, tag='cq2', name='cq2')
        ps = p64_ps.tile([C, 512], F32, tag='p64', name='cq_ps')
        nc.tensor.matmul(ps[:, :Q], lhsT=w11pnT[:], rhs=fkv[0:70, 0:Q],
                         start=True, stop=True)
        nc.scalar.activation(cq2[0:C, :], ps[:, :Q], AF.Identity,
                             bias=bqc[:])
        nc.scalar.activation(cq2[C:128, :], ps[:, :Q], AF.Identity,
                             bias=bqc[:])
        # augq rows 64-69: [-0.5 x3, qx, qy, qz] — contracts with fkv rows
        # 64-69 = [x^2, y^2, z^2, x, y, z]: score = q.k - 0.5|k|^2.
        augq = qp.tile([70, Q], F32, tag='augq', name='augq')
        nc.scalar.activation(augq[C:C + 3, :], fkv[C:C + 3, 0:Q],
                             AF.Copy, scale=0.0, bias=-0.5)
        nc.sync.dma_start(augq[C + 3:C + 6, :], fkv[C + 3:C + 6, 0:Q])
        return dict(cq2=cq2, augq=augq)

    # ---------- tile stages ----------
    def tile_scores(cb):
        sd_q, sd_k, t = cb['q'], cb['k'], cb['t']
        tsl = slice(t * 128, (t + 1) * 128)
        khat_q, khat_k = sd_q['khat'], sd_k['khat']
        fkv_k = sd_k['fkv']
        augq = cb['qd']['augq']

        sc_cos = scp.tile([128, N], F32, tag='sc', name='sc_cos')
        for j in range(N // 1024):
            ps = sc_ps.tile([128, 1024], F32, tag='sc_ps', name='sc_ps')
            for h in range(2):
                sl = slice(j * 1024 + h * 512, j * 1024 + (h + 1) * 512)
                nc.tensor.matmul(ps[:, h * 512:(h + 1) * 512],
                                 lhsT=khat_q[:, tsl], rhs=khat_k[:, sl],
                                 start=True, stop=True)
            nc.scalar.activation(sc_cos[:, j * 1024:(j + 1) * 1024],
                                 ps[:], AF.Copy)
        sc_euc = scp.tile([128, N], F32, tag='sc', name='sc_euc')
        for j in range(N // 1024):
            ps = sc_ps.tile([128, 1024], F32, tag='sc_ps', name='sc_ps')
            for h in range(2):
                sl = slice(j * 1024 + h * 512, j * 1024 + (h + 1) * 512)
                nc.tensor.matmul(ps[:, h * 512:(h + 1) * 512],
                                 lhsT=augq[C:C + 6, tsl],
                                 rhs=fkv_k[C:C + 6, sl],
                                 start=True, stop=True)
            nc.scalar.activation(sc_euc[:, j * 1024:(j + 1) * 1024],
                                 ps[:], AF.Copy)
        cb['sc_cos'], cb['sc_euc'] = sc_cos, sc_euc
        if taps is not None and cb['ci'] == 0 and t == 0:
            nc.sync.dma_start(taps['dbg_khat'], sd_k['khat'][:])
            nc.sync.dma_start(taps['dbg_fkv'], fkv_k[:])
            nc.sync.dma_start(taps['dbg_akv2'], sd_k['akv2'][:])
            nc.sync.dma_start(taps['dbg_cq2'], cb['qd']['cq2'][:])
            nc.sync.dma_start(taps['dbg_augq'], augq[:])
            nc.sync.dma_start(taps['dbg_sccos'], sc_cos[:])
            nc.sync.dma_start(taps['dbg_sceuc'], sc_euc[:])

    def tile_topk(cb):
        vals = idxp.tile([128, 16], F32, tag='vals', name='vals')
        idxu = idxp.tile([128, 16], U16, tag='idxu', name='idxu')
        nc.vector.max(vals[:, 0:8], cb['sc_cos'][:])
        nc.vector.max_index(idxu[:, 0:8], vals[:, 0:8], cb['sc_cos'][:])
        nc.vector.max(vals[:, 8:16], cb['sc_euc'][:])
        nc.vector.max_index(idxu[:, 8:16], vals[:, 8:16], cb['sc_euc'][:])
        cb['idxu'] = idxu

    def tile_post(cb):
        sd_k, t = cb['k'], cb['t']
        cq2 = cb['qd']['cq2']
        akv2 = sd_k['akv2']

        # ---- index transpose to gather layout ----
        idxf = idxp.tile([128, 16], F32, tag='idxf', name='idxf')
        nc.vector.tensor_copy(idxf[:], cb['idxu'][:])
        pst = tp_ps.tile([16, 128], F32, tag='tp', name='pst')
        nc.tensor.matmul(pst[:], lhsT=idxf[:], rhs=id128[:],
                         start=True, stop=True)
        idxrow = idxp.tile([16, 128], I16, tag='idxrow', name='idxrow')
        nc.scalar.activation(idxrow[:], pst[:], AF.Copy)
        idxT = idxp.tile([128, 64], I16, tag='idxT', name='idxT')
        for h in range(2):
            b = h * 64
            nc.sync.dma_start(idxT[b:b + 16, :],
                              idxrow[:, h * 64:(h + 1) * 64])
            nc.sync.dma_start(idxT[b + 16:b + 32, :], idxT[b:b + 16, :])
            nc.sync.dma_start(idxT[b + 32:b + 64, :], idxT[b:b + 32, :])

        # ---- gathers: neighbors + per-query cq expansion ----
        ag = mlpp.tile([128, 1024], F32, tag='ag', name='ag', bufs=2)
        nc.gpsimd.ap_gather(ag[:], akv2[:], idxT[:], channels=128,
                            num_elems=N, d=1, num_idxs=1024)
        cqs = mlpp.tile([128, 1024], F32, tag='cqs', name='cqs')
        nc.gpsimd.ap_gather(cqs[:], cq2[:], cqidx[:, t * 64:(t + 1) * 64],
                            channels=128, num_elems=Q, d=1, num_idxs=1024)
        if taps is not None and cb['ci'] == 0 and t == 0:
            nc.sync.dma_start(taps['dbg_idxf'], idxf[:])
            nc.sync.dma_start(taps['dbg_ag'], ag[:])
            nc.sync.dma_start(taps['dbg_cqs'], cqs[:])

        # ---- layer 0: add cq, leaky (-> fp16) ----
        nc.gpsimd.tensor_tensor(ag[:], ag[:], cqs[:], op=ALU.add)
        n1 = mlpp.tile([128, 1024], F16, tag='n1', name='n1')
        nc.scalar.activation(n1[:], ag[:], AF.Prelu, alpha=LEAKY)

        # ---- layers 1, 2 (fp16, stacked weights) ----
        cur = n1
        for li, (w, bias) in enumerate(((wm1stk, bm1s), (wm2stk, bm2s))):
            ps = mlp_ps.tile([128, 1024], F32, tag='mlp', name='mlp_ps')
            for h in range(2):
                hs = slice(h * 512, (h + 1) * 512)
                nc.tensor.matmul(ps[:, hs], lhsT=w[:], rhs=cur[:, hs],
                                 start=True, stop=True)
            ht = mlpp.tile([128, 1024], F16, tag=f'h{li + 1}',
                           name=f'h{li + 1}')
            nc.scalar.activation(ht[:], ps[:], AF.Prelu, bias=bias[:],
                                 alpha=LEAKY)
            cur = ht
        if taps is not None and cb['ci'] == 0 and t == 0:
            nc.sync.dma_start(taps['dbg_h2'], cur[:])

        # ---- maxpool over NS neighbors (pair tree on gpsimd; adjacent
        # slots pair up as stride-2 2D APs) ----
        width = NS
        while width > 1:
            w2 = width // 2
            if w2 == 1:
                nxt = cb['outc']
                dst = nxt[:, t * 64:(t + 1) * 64]
            else:
                nxt = mlpp.tile([128, 64 * w2], F16, tag=f'mp{w2}',
                                name=f'mp{w2}')
                dst = nxt[:]
            v = cur[:].rearrange('c (w two) -> c w two', two=2)
            nc.vector.tensor_tensor(dst, v[:, :, 0], v[:, :, 1],
                                    op=ALU.max)
            cur = nxt
            width = w2

    def emit_out(cb):
        # one contiguous DMA per combo: [128, 256] -> out[b, h, c, :]
        outap = cb['outap']
        base = outap.offset + cb['bi'] * 2 * C * 64 * NT
        dst = bass.AP(outap.tensor, base,
                      [[C * 64 * NT, 2], [64 * NT, C], [1, 64 * NT]])
        nc.sync.dma_start(dst, cb['outc'][:])

    # ---------- main loop (software pipelined, incl. batch-level) ----------
    pending = None
    pending_out = None
    ci = 0
    sd_next = {1: key_prep(0, 1), 2: key_prep(0, 2)}
    for bi in range(B):
        sd = sd_next
        for outap, qs, ks in ((out1, 1, 2), (out2, 2, 1)):
            qd = query_prep(sd[qs])
            outc = outp.tile([128, 64 * NT], F32, tag='outc', name='outc')
            for t in range(NT):
                cb = dict(q=sd[qs], k=sd[ks], qd=qd, t=t, bi=bi,
                          outap=outap, outc=outc, ci=ci)
                tile_scores(cb)
                if pending is not None:
                    tile_post(pending)
                if pending_out is not None and ci > pending_out[0] \
                        and t == 1:
                    emit_out(pending_out[1])
                    pending_out = None
                tile_topk(cb)
                pending = cb
            pending_out = (ci, cb)
            ci += 1
        if bi + 1 < B:
            # prefetch next batch's key prep behind this batch's tail
            sd_next = {1: key_prep(bi + 1, 1), 2: key_prep(bi + 1, 2)}
    tile_post(pending)
    emit_out(pending_out[1])
    ctx.close()


# ======================= host side =======================

_CACHED = {}


def _get_nc():
    if 'nc' not in _CACHED:
        _CACHED['nc'] = build_nc()
    return _CACHED['nc']


def make_in_maps(pc1, pc2, feat1, feat2, knn1, knn2,
                 W_t11, b_t11, W_t22, b_t22, W_pos, b_pos,
                 W_m1, b_m1, W_m2, b_m2):
    f32, f16 = np.float32, np.float16
    W_t11 = np.asarray(W_t11, f32); W_t22 = np.asarray(W_t22, f32)
    W_pos = np.asarray(W_pos, f32)
    W_m1 = np.asarray(W_m1, f32); W_m2 = np.asarray(W_m2, f32)

    z3 = np.zeros((3, C), f32)
    w22pT = np.vstack([W_t22.T, z3, W_pos.T]).astype(f16)        # [70, 64]
    w11pnT = np.vstack([W_t11.T, z3, -W_pos.T]).astype(f32)      # [70, 64]
    z = np.zeros((C, C), f32)
    wm1stk = np.block([[W_m1.T, z], [z, W_m1.T]]).astype(f16)    # [128,128]
    wm2stk = np.block([[W_m2.T, z], [z, W_m2.T]]).astype(f16)
    b22 = np.asarray(b_t22, f32).reshape(C, 1)
    bqc = (np.asarray(b_t11, f32) + np.asarray(b_pos, f32)).reshape(C, 1)
    bm1s = np.tile(np.asarray(b_m1, f32).reshape(C, 1), (2, 1))
    bm2s = np.tile(np.asarray(b_m2, f32).reshape(C, 1), (2, 1))

    # cqidx[16g+s, t*64+q] = t*128 + 64*(g>=4) + q: groups 0-3 expand the
    # first 64 queries of tile t, groups 4-7 the second 64.
    cqidx = np.zeros((128, 64 * NT), np.int16)
    for g in range(8):
        h = g // 4
        for t in range(NT):
            cqidx[16 * g:16 * (g + 1), t * 64:(t + 1) * 64] = \
                t * 128 + 64 * h + np.arange(64, dtype=np.int16)[None, :]

    def build_fkv(feat, pc):
        b, _, n = feat.shape
        fkv = np.zeros((b, FKR, n), f32)
        fkv[:, 0:C] = feat
        fkv[:, C + 3:C + 6] = pc
        fkv[:, 96:99] = pc
        return fkv

    fkv1 = build_fkv(np.asarray(feat1, f32), np.asarray(pc1, f32))
    fkv2 = build_fkv(np.asarray(feat2, f32), np.asarray(pc2, f32))
    knn1 = np.asarray(knn1, f32)
    knn2 = np.asarray(knn2, f32)

    base = {
        'w22pT': w22pT, 'w11pnT': w11pnT,
        'wm1stk': wm1stk, 'wm2stk': wm2stk,
        'b22': b22, 'bqc': bqc, 'bm1s': bm1s, 'bm2s': bm2s,
        'id128': np.eye(128, dtype=f32),
        'cqidx': cqidx,
    }
    in_maps = []
    for c in range(NCORES):
        m = dict(base)
        r = -c * Q
        m['knn1'] = np.ascontiguousarray(np.roll(knn1, r, axis=2))
        m['knn2'] = np.ascontiguousarray(np.roll(knn2, r, axis=2))
        m['fkv1'] = np.ascontiguousarray(np.roll(fkv1, r, axis=2))
        m['fkv2'] = np.ascontiguousarray(np.roll(fkv2, r, axis=2))
        in_maps.append(m)
    return in_maps


def _unstack_out(res, name):
    # per-core out [B, 2, C, 256] -> [B, C, 512] block, concat on queries
    blocks = []
    for c in range(NCORES):
        o = res.results[c][name]           # [B, 2, C, 256]
        o = o.reshape(B, 2, C, NT, 64)     # [b, h, c, t, q]
        o = o.transpose(0, 2, 3, 1, 4)     # [b, c, t, h, q]
        blocks.append(o.reshape(B, C, Q))
    return np.concatenate(blocks, axis=2)


def kernel(pc1, pc2, feat1, feat2, knn1, knn2,
           W_t11, b_t11, W_t22, b_t22, W_pos, b_pos,
           W_m1, b_m1, W_m2, b_m2):
    from concourse.bass_utils import run_bass_kernel_spmd
    nc = _get_nc()
    in_maps = make_in_maps(pc1, pc2, feat1, feat2, knn1, knn2,
                           W_t11, b_t11, W_t22, b_t22, W_pos, b_pos,
                           W_m1, b_m1, W_m2, b_m2)
    res = run_bass_kernel_spmd(nc, in_maps, core_ids=list(range(NCORES)))
    return _unstack_out(res, 'out1'), _unstack_out(res, 'out2')


# revision 54
# speedup vs baseline: 58.9030x; 1.0079x over previous
"""Trainium2 Bass kernel for nn_BidirectionalLayerFeatCosine (retrieval_knn).

Strategy: shard the 4096 query points across 8 NeuronCores (512 each); keys
are replicated.  Host rolls the key axis per core so each core's query block
is always columns 0:512 (SPMD-clean static slices).

Per core, per batch, per side: ONE wide DMA loads [feat; _; pc; pc-dup]
(fkv tile, rows 0-63 / 64-66(pc^2 target) / 67-69 / 96-98); knn is loaded in
chunks and normalized exactly in fp32 via gpsimd partition_all_reduce + ACT
sqrt + DVE reciprocal + gpsimd multiply -> khat (query side is a slice).
akv = W22@feat + Wpos@pc + b22 via one fp16 70-row matmul per chunk,
replicated to 128 partitions for the stacked gather.  Scores (cos + euclid)
stay exact fp32 on the PE; top-8 via DVE max8/find_index8; ap_gather pulls
neighbors (and a second static-index gather expands cq); the fp16 MLP uses
block-diagonal stacked weights; maxpool pair-tree; per-combo output
accumulation with one contiguous DMA emitted a combo late.
"""
import sys

for _p in ('/opt/trn_rl_repo',):
    if _p not in sys.path:
        sys.path.insert(0, _p)

import numpy as np
import concourse.bass as bass
import concourse.tile as tile
from concourse import bacc, mybir, bass_isa

F32 = mybir.dt.float32
F16 = mybir.dt.float16
I16 = mybir.dt.int16
U16 = mybir.dt.uint16
AF = mybir.ActivationFunctionType
ALU = mybir.AluOpType
ROP = bass_isa.ReduceOp

B, N, C, NS = 2, 4096, 64, 16
NCORES = 8
Q = N // NCORES           # queries per core per combo (512)
NT = Q // 128             # query tiles per combo (4)
LEAKY = 0.1
EPS = 1e-8
FKR = 99                  # fkv tile rows


def build_nc(debug_taps=False):
    nc = bacc.Bacc("TRN2", num_devices=NCORES, debug=False)

    def din(name, shape, dt=F32):
        return nc.dram_tensor(name, list(shape), dt, kind="ExternalInput").ap()

    ins = {
        'knn1': din('knn1', (B, C, N)),
        'knn2': din('knn2', (B, C, N)),
        'fkv1': din('fkv1', (B, FKR, N)),
        'fkv2': din('fkv2', (B, FKR, N)),
        'w22pT': din('w22pT', (70, C), F16),
        'w11pnT': din('w11pnT', (70, C)),
        'wm1stk': din('wm1stk', (128, 128), F16),
        'wm2stk': din('wm2stk', (128, 128), F16),
        'b22': din('b22', (C, 1)),
        'bqc': din('bqc', (C, 1)),
        'bm1s': din('bm1s', (128, 1)),
        'bm2s': din('bm2s', (128, 1)),
        'id128': din('id128', (128, 128)),
        'cqidx': din('cqidx', (128, 64 * NT), I16),
    }
    # out[b, h, c, t*64+q] = feat_new[b, c, 512*core + t*128 + 64*h + q]
    out1 = nc.dram_tensor('out1', [B, 2, C, 64 * NT], F32,
                          kind="ExternalOutput").ap()
    out2 = nc.dram_tensor('out2', [B, 2, C, 64 * NT], F32,
                          kind="ExternalOutput").ap()
    taps = None
    if debug_taps:
        taps = {nm: nc.dram_tensor(nm, list(sh), dt, kind="ExternalOutput").ap()
                for nm, sh, dt in [
                    ('dbg_khat', (C, N), F32),
                    ('dbg_fkv', (FKR, N), F32),
                    ('dbg_akv2', (128, N), F32),
                    ('dbg_cq2', (128, Q), F32),
                    ('dbg_augq', (70, Q), F32),
                    ('dbg_sccos', (128, N), F32),
                    ('dbg_sceuc', (128, N), F32),
                    ('dbg_idxf', (128, 16), F32),
                    ('dbg_ag', (128, 1024), F32),
                    ('dbg_cqs', (128, 1024), F32),
                    ('dbg_h2', (128, 1024), F16),
                ]}

    with tile.TileContext(nc) as tc:
        _body(tc, ins, out1, out2, taps)
    nc.compile()
    return nc


def _body(tc, ins, out1, out2, taps=None):
    nc = tc.nc
    from contextlib import ExitStack
    ctx = ExitStack()

    pool = lambda name, bufs, space='SBUF': ctx.enter_context(
        tc.tile_pool(name=name, bufs=bufs, space=space))

    consts = pool('consts', 1)
    inp = pool('inputs', 2)        # fkv tiles (rotate across sides/batches)
    prep = pool('prep', 2)         # chunked scratch for normalization
    keyp = pool('keyprep', 2)      # khat / akv2 (both sides live)
    f16p = pool('f16', 1)
    qp = pool('qside', 2)          # cq2 / augq per combo
    scp = pool('scores', 2)        # [128, 4096] score rows
    idxp = pool('idx', 2)          # vals/idx tiles per tile
    mlpp = pool('mlp', 1)
    outp = pool('out', 2)

    sc_ps = ctx.enter_context(tc.tile_pool(name='sc_ps', bufs=2, space='PSUM'))
    mlp_ps = ctx.enter_context(tc.tile_pool(name='mlp_ps', bufs=1, space='PSUM'))
    p64_ps = ctx.enter_context(tc.tile_pool(name='p64_ps', bufs=1, space='PSUM'))
    tp_ps = ctx.enter_context(tc.tile_pool(name='tp_ps', bufs=1, space='PSUM'))

    # ---- constants ----
    def cload(name, shape, dt=F32):
        t = consts.tile(list(shape), dt, tag=name, name=name)
        nc.sync.dma_start(t[:], ins[name])
        return t

    w22pT = cload('w22pT', (70, C), F16)
    w11pnT = cload('w11pnT', (70, C))
    wm1stk = cload('wm1stk', (128, 128), F16)
    wm2stk = cload('wm2stk', (128, 128), F16)
    b22 = cload('b22', (C, 1))
    bqc = cload('bqc', (C, 1))
    bm1s = cload('bm1s', (128, 1))
    bm2s = cload('bm2s', (128, 1))
    id128 = cload('id128', (128, 128))
    cqidx = cload('cqidx', (128, 64 * NT), I16)
    eps64 = consts.tile([C, 1], F32, tag='eps64', name='eps64')
    nc.vector.memset(eps64[:], EPS)

    # ---------- per (batch, side) key prep ----------
    CH = 512                       # normalization chunk width

    def key_khat(bi, side):
        # khat = knn / sqrt(colsum(knn^2) + eps), exact fp32 matching the
        # reference's rounding (sqrt of biased sum, then reciprocal —
        # reordering these flips near-tie neighbor selections).
        knn_d = ins['knn1'] if side == 1 else ins['knn2']
        knn = prep.tile([C, N], F32, tag='knn', name='knn', bufs=1)
        nc.sync.dma_start(knn[:], knn_d[bi])
        khat = keyp.tile([C, N], F32, tag='khat', name='khat')
        for j in range(N // CH):
            sl = slice(j * CH, (j + 1) * CH)
            ksq = prep.tile([C, CH], F32, tag='scrA', name='ksq')
            nc.gpsimd.tensor_tensor(ksq[:], knn[:, sl], knn[:, sl],
                                    op=ALU.mult)
            ssb = prep.tile([C, CH], F32, tag='scrB', name='ssb')
            nc.gpsimd.partition_all_reduce(ssb[:], ksq[:], channels=C,
                                           reduce_op=ROP.add)
            nc.scalar.activation(ssb[:], ssb[:], AF.Sqrt, bias=eps64[:])
            rinv = prep.tile([C, CH], F32, tag='scrA', name='rinv')
            nc.vector.reciprocal(rinv[:], ssb[:])
            nc.gpsimd.tensor_tensor(khat[:, sl], knn[:, sl], rinv[:],
                                    op=ALU.mult)
        return khat

    def key_rest(bi, side, khat):
        fkv_d = ins['fkv1'] if side == 1 else ins['fkv2']
        # fkv rows: 0-63 feat, 64-66 pc^2 (computed), 67-69 pc, 96-98 pc.
        fkv = inp.tile([FKR, N], F32, tag='fkv', name='fkv')
        nc.sync.dma_start(fkv[:], fkv_d[bi])
        nc.scalar.activation(fkv[C:C + 3, :], fkv[96:99, :], AF.Square)

        # akv2 = [W22; 0; Wpos] @ fkv[0:70] + b22, replicated to 128 parts
        fkv16 = f16p.tile([70, N], F16, tag='fkv16', name='fkv16')
        nc.gpsimd.tensor_copy(fkv16[:], fkv[0:70, :])
        akv2 = keyp.tile([128, N], F32, tag='akv2', name='akv2')
        for kb in range(N // 512):
            sl = slice(kb * 512, (kb + 1) * 512)
            ps = p64_ps.tile([C, 512], F32, tag='p64', name='akv_ps')
            nc.tensor.matmul(ps[:], lhsT=w22pT[:], rhs=fkv16[:, sl],
                             start=True, stop=True)
            nc.scalar.activation(akv2[0:C, sl], ps[:], AF.Identity,
                                 bias=b22[:])
        nc.sync.dma_start(akv2[C:128, :], akv2[0:C, :])
        return dict(fkv=fkv, khat=khat, akv2=akv2)

    def key_prep_batch(bi):
        kh1 = key_khat(bi, 1)
        kh2 = key_khat(bi, 2)
        return {1: key_rest(bi, 1, kh1), 2: key_rest(bi, 2, kh2)}

    # ---------- per-combo query prep ----------
    def query_prep(sd_q):
        fkv = sd_q['fkv']
        # cq = W11@feat_q - Wpos@pc_q + (b11 + bpos), stacked to 128 rows
        cq2 = qp.tile([128, Q], F32, tag='cq2', name='cq2')
        ps = p64_ps.tile([C, 512], F32, tag='p64', name='cq_ps')
        nc.tensor.matmul(ps[:, :Q], lhsT=w11pnT[:], rhs=fkv[0:70, 0:Q],
                         start=True, stop=True)
        nc.scalar.activation(cq2[0:C, :], ps[:, :Q], AF.Identity,
                             bias=bqc[:])
        nc.scalar.activation(cq2[C:128, :], ps[:, :Q], AF.Identity,
                             bias=bqc[:])
        # augq rows 64-69: [-0.5 x3, qx, qy, qz] — contracts with fkv rows
        # 64-69 = [x^2, y^2, z^2, x, y, z]: score = q.k - 0.5|k|^2.
        augq = qp.tile([70, Q], F32, tag='augq', name='augq')
        nc.scalar.activation(augq[C:C + 3, :], fkv[C:C + 3, 0:Q],
                             AF.Copy, scale=0.0, bias=-0.5)
        nc.sync.dma_start(augq[C + 3:C + 6, :], fkv[C + 3:C + 6, 0:Q])
        return dict(cq2=cq2, augq=augq)

    # ---------- tile stages ----------
    def tile_scores(cb):
        sd_q, sd_k, t = cb['q'], cb['k'], cb['t']
        tsl = slice(t * 128, (t + 1) * 128)
        khat_q, khat_k = sd_q['khat'], sd_k['khat']
        fkv_k = sd_k['fkv']
        augq = cb['qd']['augq']

        sc_cos = scp.tile([128, N], F32, tag='sc', name='sc_cos')
        for j in range(N // 1024):
            ps = sc_ps.tile([128, 1024], F32, tag='sc_ps', name='sc_ps')
            for h in range(2):
                sl = slice(j * 1024 + h * 512, j * 1024 + (h + 1) * 512)
                nc.tensor.matmul(ps[:, h * 512:(h + 1) * 512],
                                 lhsT=khat_q[:, tsl], rhs=khat_k[:, sl],
                                 start=True, stop=True)
            nc.scalar.activation(sc_cos[:, j * 1024:(j + 1) * 1024],
                                 ps[:], AF.Copy)
        sc_euc = scp.tile([128, N], F32, tag='sc', name='sc_euc')
        for j in range(N // 1024):
            ps = sc_ps.tile([128, 1024], F32, tag='sc_ps', name='sc_ps')
            for h in range(2):
                sl = slice(j * 1024 + h * 512, j * 1024 + (h + 1) * 512)
                nc.tensor.matmul(ps[:, h * 512:(h + 1) * 512],
                                 lhsT=augq[C:C + 6, tsl],
                                 rhs=fkv_k[C:C + 6, sl],
                                 start=True, stop=True)
            nc.scalar.activation(sc_euc[:, j * 1024:(j + 1) * 1024],
                                 ps[:], AF.Copy)
        cb['sc_cos'], cb['sc_euc'] = sc_cos, sc_euc
        if taps is not None and cb['ci'] == 0 and t == 0:
            nc.sync.dma_start(taps['dbg_khat'], sd_k['khat'][:])
            nc.sync.dma_start(taps['dbg_fkv'], fkv_k[:])
            nc.sync.dma_start(taps['dbg_akv2'], sd_k['akv2'][:])
            nc.sync.dma_start(taps['dbg_cq2'], cb['qd']['cq2'][:])
            nc.sync.dma_start(taps['dbg_augq'], augq[:])
            nc.sync.dma_start(taps['dbg_sccos'], sc_cos[:])
            nc.sync.dma_start(taps['dbg_sceuc'], sc_euc[:])

    def tile_topk(cb):
        vals = idxp.tile([128, 16], F32, tag='vals', name='vals')
        idxu = idxp.tile([128, 16], U16, tag='idxu', name='idxu')
        nc.vector.max(vals[:, 0:8], cb['sc_cos'][:])
        nc.vector.max_index(idxu[:, 0:8], vals[:, 0:8], cb['sc_cos'][:])
        nc.vector.max(vals[:, 8:16], cb['sc_euc'][:])
        nc.vector.max_index(idxu[:, 8:16], vals[:, 8:16], cb['sc_euc'][:])
        cb['idxu'] = idxu

    def tile_post(cb):
        sd_k, t = cb['k'], cb['t']
        cq2 = cb['qd']['cq2']
        akv2 = sd_k['akv2']

        # ---- index transpose to gather layout ----
        idxf = idxp.tile([128, 16], F32, tag='idxf', name='idxf')
        nc.vector.tensor_copy(idxf[:], cb['idxu'][:])
        pst = tp_ps.tile([16, 128], F32, tag='tp', name='pst')
        nc.tensor.matmul(pst[:], lhsT=idxf[:], rhs=id128[:],
                         start=True, stop=True)
        idxrow = idxp.tile([16, 128], I16, tag='idxrow', name='idxrow')
        nc.scalar.activation(idxrow[:], pst[:], AF.Copy)
        idxT = idxp.tile([128, 64], I16, tag='idxT', name='idxT')
        for h in range(2):
            b = h * 64
            nc.sync.dma_start(idxT[b:b + 16, :],
                              idxrow[:, h * 64:(h + 1) * 64])
            nc.sync.dma_start(idxT[b + 16:b + 32, :], idxT[b:b + 16, :])
            nc.sync.dma_start(idxT[b + 32:b + 64, :], idxT[b:b + 32, :])

        # ---- gathers: neighbors + per-query cq expansion ----
        ag = mlpp.tile([128, 1024], F32, tag='ag', name='ag', bufs=2)
        nc.gpsimd.ap_gather(ag[:], akv2[:], idxT[:], channels=128,
                            num_elems=N, d=1, num_idxs=1024)
        cqs = mlpp.tile([128, 1024], F32, tag='cqs', name='cqs')
        nc.gpsimd.ap_gather(cqs[:], cq2[:], cqidx[:, t * 64:(t + 1) * 64],
                            channels=128, num_elems=Q, d=1, num_idxs=1024)
        if taps is not None and cb['ci'] == 0 and t == 0:
            nc.sync.dma_start(taps['dbg_idxf'], idxf[:])
            nc.sync.dma_start(taps['dbg_ag'], ag[:])
            nc.sync.dma_start(taps['dbg_cqs'], cqs[:])

        # ---- layer 0: add cq, leaky (-> fp16) ----
        nc.gpsimd.tensor_tensor(ag[:], ag[:], cqs[:], op=ALU.add)
        n1 = mlpp.tile([128, 1024], F16, tag='n1', name='n1')
        nc.scalar.activation(n1[:], ag[:], AF.Prelu, alpha=LEAKY)

        # ---- layers 1, 2 (fp16, stacked weights) ----
        cur = n1
        for li, (w, bias) in enumerate(((wm1stk, bm1s), (wm2stk, bm2s))):
            ps = mlp_ps.tile([128, 1024], F32, tag='mlp', name='mlp_ps')
            for h in range(2):
                hs = slice(h * 512, (h + 1) * 512)
                nc.tensor.matmul(ps[:, hs], lhsT=w[:], rhs=cur[:, hs],
                                 start=True, stop=True)
            ht = mlpp.tile([128, 1024], F16, tag=f'h{li + 1}',
                           name=f'h{li + 1}')
            nc.scalar.activation(ht[:], ps[:], AF.Prelu, bias=bias[:],
                                 alpha=LEAKY)
            cur = ht
        if taps is not None and cb['ci'] == 0 and t == 0:
            nc.sync.dma_start(taps['dbg_h2'], cur[:])

        # ---- maxpool over NS neighbors (pair tree on gpsimd; adjacent
        # slots pair up as stride-2 2D APs) ----
        width = NS
        while width > 1:
            w2 = width // 2
            if w2 == 1:
                nxt = cb['outc']
                dst = nxt[:, t * 64:(t + 1) * 64]
            else:
                nxt = mlpp.tile([128, 64 * w2], F16, tag=f'mp{w2}',
                                name=f'mp{w2}')
                dst = nxt[:]
            v = cur[:].rearrange('c (w two) -> c w two', two=2)
            nc.vector.tensor_tensor(dst, v[:, :, 0], v[:, :, 1],
                                    op=ALU.max)
            cur = nxt
            width = w2

    def emit_out(cb):
        # one contiguous DMA per combo: [128, 256] -> out[b, h, c, :]
        outap = cb['outap']
        base = outap.offset + cb['bi'] * 2 * C * 64 * NT
        dst = bass.AP(outap.tensor, base,
                      [[C * 64 * NT, 2], [64 * NT, C], [1, 64 * NT]])
        nc.sync.dma_start(dst, cb['outc'][:])

    # ---------- main loop (software pipelined, incl. batch-level) ----------
    pending = None
    pending_out = None
    ci = 0
    sd_next = key_prep_batch(0)
    for bi in range(B):
        sd = sd_next
        for outap, qs, ks in ((out1, 1, 2), (out2, 2, 1)):
            qd = query_prep(sd[qs])
            outc = outp.tile([128, 64 * NT], F32, tag='outc', name='outc')
            for t in range(NT):
                cb = dict(q=sd[qs], k=sd[ks], qd=qd, t=t, bi=bi,
                          outap=outap, outc=outc, ci=ci)
                tile_scores(cb)
                if pending is not None:
                    tile_post(pending)
                if pending_out is not None and ci > pending_out[0] \
                        and t == 1:
                    emit_out(pending_out[1])
                    pending_out = None
                tile_topk(cb)
                pending = cb
            pending_out = (ci, cb)
            ci += 1
        if bi + 1 < B:
            # prefetch next batch's key prep behind this batch's tail
            sd_next = key_prep_batch(bi + 1)
    tile_post(pending)
    emit_out(pending_out[1])
    ctx.close()


# ======================= host side =======================

_CACHED = {}


def _get_nc():
    if 'nc' not in _CACHED:
        _CACHED['nc'] = build_nc()
    return _CACHED['nc']


def make_in_maps(pc1, pc2, feat1, feat2, knn1, knn2,
                 W_t11, b_t11, W_t22, b_t22, W_pos, b_pos,
                 W_m1, b_m1, W_m2, b_m2):
    f32, f16 = np.float32, np.float16
    W_t11 = np.asarray(W_t11, f32); W_t22 = np.asarray(W_t22, f32)
    W_pos = np.asarray(W_pos, f32)
    W_m1 = np.asarray(W_m1, f32); W_m2 = np.asarray(W_m2, f32)

    z3 = np.zeros((3, C), f32)
    w22pT = np.vstack([W_t22.T, z3, W_pos.T]).astype(f16)        # [70, 64]
    w11pnT = np.vstack([W_t11.T, z3, -W_pos.T]).astype(f32)      # [70, 64]
    z = np.zeros((C, C), f32)
    wm1stk = np.block([[W_m1.T, z], [z, W_m1.T]]).astype(f16)    # [128,128]
    wm2stk = np.block([[W_m2.T, z], [z, W_m2.T]]).astype(f16)
    b22 = np.asarray(b_t22, f32).reshape(C, 1)
    bqc = (np.asarray(b_t11, f32) + np.asarray(b_pos, f32)).reshape(C, 1)
    bm1s = np.tile(np.asarray(b_m1, f32).reshape(C, 1), (2, 1))
    bm2s = np.tile(np.asarray(b_m2, f32).reshape(C, 1), (2, 1))

    # cqidx[16g+s, t*64+q] = t*128 + 64*(g>=4) + q: groups 0-3 expand the
    # first 64 queries of tile t, groups 4-7 the second 64.
    cqidx = np.zeros((128, 64 * NT), np.int16)
    for g in range(8):
        h = g // 4
        for t in range(NT):
            cqidx[16 * g:16 * (g + 1), t * 64:(t + 1) * 64] = \
                t * 128 + 64 * h + np.arange(64, dtype=np.int16)[None, :]

    def build_fkv(feat, pc):
        b, _, n = feat.shape
        fkv = np.zeros((b, FKR, n), f32)
        fkv[:, 0:C] = feat
        fkv[:, C + 3:C + 6] = pc
        fkv[:, 96:99] = pc
        return fkv

    fkv1 = build_fkv(np.asarray(feat1, f32), np.asarray(pc1, f32))
    fkv2 = build_fkv(np.asarray(feat2, f32), np.asarray(pc2, f32))
    knn1 = np.asarray(knn1, f32)
    knn2 = np.asarray(knn2, f32)

    base = {
        'w22pT': w22pT, 'w11pnT': w11pnT,
        'wm1stk': wm1stk, 'wm2stk': wm2stk,
        'b22': b22, 'bqc': bqc, 'bm1s': bm1s, 'bm2s': bm2s,
        'id128': np.eye(128, dtype=f32),
        'cqidx': cqidx,
    }
    in_maps = []
    for c in range(NCORES):
        m = dict(base)
        r = -c * Q
        m['knn1'] = np.ascontiguousarray(np.roll(knn1, r, axis=2))
        m['knn2'] = np.ascontiguousarray(np.roll(knn2, r, axis=2))
        m['fkv1'] = np.ascontiguousarray(np.roll(fkv1, r, axis=2))
        m['fkv2'] = np.ascontiguousarray(np.roll(fkv2, r, axis=2))
        in_maps.append(m)
    return in_maps


def _unstack_out(res, name):
    # per-core out [B, 2, C, 256] -> [B, C, 512] block, concat on queries
    blocks = []
    for c in range(NCORES):
        o = res.results[c][name]           # [B, 2, C, 256]
        o = o.reshape(B, 2, C, NT, 64)     # [b, h, c, t, q]
        o = o.transpose(0, 2, 3, 1, 4)     # [b, c, t, h, q]
        blocks.append(o.reshape(B, C, Q))
    return np.concatenate(blocks, axis=2)


def kernel(pc1, pc2, feat1, feat2, knn1, knn2,
           W_t11, b_t11, W_t22, b_t22, W_pos, b_pos,
           W_m1, b_m1, W_m2, b_m2):
    from concourse.bass_utils import run_bass_kernel_spmd
    nc = _get_nc()
    in_maps = make_in_maps(pc1, pc2, feat1, feat2, knn1, knn2,
                           W_t11, b_t11, W_t22, b_t22, W_pos, b_pos,
                           W_m1, b_m1, W_m2, b_m2)
    res = run_bass_kernel_spmd(nc, in_maps, core_ids=list(range(NCORES)))
    return _unstack_out(res, 'out1'), _unstack_out(res, 'out2')


# revision 56
# speedup vs baseline: 60.0452x; 1.0194x over previous
"""Trainium2 Bass kernel for nn_BidirectionalLayerFeatCosine (retrieval_knn).

Strategy: shard the 4096 query points across 8 NeuronCores (512 each); keys
are replicated.  Host rolls the key axis per core so each core's query block
is always columns 0:512 (SPMD-clean static slices).

Per core, per batch, per side: ONE wide DMA loads [feat; _; pc; pc-dup]
(fkv tile, rows 0-63 / 64-66(pc^2 target) / 67-69 / 96-98); knn is loaded in
chunks and normalized exactly in fp32 via gpsimd partition_all_reduce + ACT
sqrt + DVE reciprocal + gpsimd multiply -> khat (query side is a slice).
akv = W22@feat + Wpos@pc + b22 via one fp16 70-row matmul per chunk,
replicated to 128 partitions for the stacked gather.  Scores (cos + euclid)
stay exact fp32 on the PE; top-8 via DVE max8/find_index8; ap_gather pulls
neighbors (and a second static-index gather expands cq); the fp16 MLP uses
block-diagonal stacked weights; maxpool pair-tree; per-combo output
accumulation with one contiguous DMA emitted a combo late.
"""
import sys

for _p in ('/opt/trn_rl_repo',):
    if _p not in sys.path:
        sys.path.insert(0, _p)

import numpy as np
import concourse.bass as bass
import concourse.tile as tile
from concourse import bacc, mybir, bass_isa

F32 = mybir.dt.float32
F16 = mybir.dt.float16
I16 = mybir.dt.int16
U16 = mybir.dt.uint16
AF = mybir.ActivationFunctionType
ALU = mybir.AluOpType
ROP = bass_isa.ReduceOp

B, N, C, NS = 2, 4096, 64, 16
NCORES = 8
Q = N // NCORES           # queries per core per combo (512)
NT = Q // 128             # query tiles per combo (4)
LEAKY = 0.1
EPS = 1e-8
FKR = 99                  # fkv tile rows


def build_nc(debug_taps=False):
    nc = bacc.Bacc("TRN2", num_devices=NCORES, debug=False)

    def din(name, shape, dt=F32):
        return nc.dram_tensor(name, list(shape), dt, kind="ExternalInput").ap()

    ins = {
        'knn1': din('knn1', (B, C, N)),
        'knn2': din('knn2', (B, C, N)),
        'fkv1': din('fkv1', (B, FKR, N)),
        'fkv2': din('fkv2', (B, FKR, N)),
        'w22pT': din('w22pT', (70, C), F16),
        'w11pnT': din('w11pnT', (70, C)),
        'wm1stk': din('wm1stk', (128, 128), F16),
        'wm2stk': din('wm2stk', (128, 128), F16),
        'b22': din('b22', (C, 1)),
        'bqc': din('bqc', (C, 1)),
        'bm1s': din('bm1s', (128, 1)),
        'bm2s': din('bm2s', (128, 1)),
        'id128': din('id128', (128, 128)),
        'cqidx': din('cqidx', (128, 64 * NT), I16),
    }
    # out[b, h, c, t*64+q] = feat_new[b, c, 512*core + t*128 + 64*h + q]
    out1 = nc.dram_tensor('out1', [B, 2, C, 64 * NT], F32,
                          kind="ExternalOutput").ap()
    out2 = nc.dram_tensor('out2', [B, 2, C, 64 * NT], F32,
                          kind="ExternalOutput").ap()
    taps = None
    if debug_taps:
        taps = {nm: nc.dram_tensor(nm, list(sh), dt, kind="ExternalOutput").ap()
                for nm, sh, dt in [
                    ('dbg_khat', (C, N), F32),
                    ('dbg_fkv', (FKR, N), F32),
                    ('dbg_akv2', (128, N), F32),
                    ('dbg_cq2', (128, Q), F32),
                    ('dbg_augq', (70, Q), F32),
                    ('dbg_sccos', (128, N), F32),
                    ('dbg_sceuc', (128, N), F32),
                    ('dbg_idxf', (128, 16), F32),
                    ('dbg_ag', (128, 1024), F32),
                    ('dbg_cqs', (128, 1024), F32),
                    ('dbg_h2', (128, 1024), F16),
                ]}

    with tile.TileContext(nc) as tc:
        _body(tc, ins, out1, out2, taps)
    nc.compile()
    return nc


def _body(tc, ins, out1, out2, taps=None):
    nc = tc.nc
    from contextlib import ExitStack
    ctx = ExitStack()

    pool = lambda name, bufs, space='SBUF': ctx.enter_context(
        tc.tile_pool(name=name, bufs=bufs, space=space))

    consts = pool('consts', 1)
    inp = pool('inputs', 2)        # fkv tiles (rotate across sides/batches)
    prep = pool('prep', 2)         # chunked scratch for normalization
    keyp = pool('keyprep', 2)      # khat / akv2 (both sides live)
    f16p = pool('f16', 1)
    qp = pool('qside', 2)          # cq2 / augq per combo
    scp = pool('scores', 2)        # [128, 4096] score rows
    idxp = pool('idx', 2)          # vals/idx tiles per tile
    mlpp = pool('mlp', 1)
    outp = pool('out', 2)

    sc_ps = ctx.enter_context(tc.tile_pool(name='sc_ps', bufs=2, space='PSUM'))
    mlp_ps = ctx.enter_context(tc.tile_pool(name='mlp_ps', bufs=1, space='PSUM'))
    p64_ps = ctx.enter_context(tc.tile_pool(name='p64_ps', bufs=1, space='PSUM'))
    tp_ps = ctx.enter_context(tc.tile_pool(name='tp_ps', bufs=1, space='PSUM'))

    # ---- constants ----
    def cload(name, shape, dt=F32):
        t = consts.tile(list(shape), dt, tag=name, name=name)
        nc.sync.dma_start(t[:], ins[name])
        return t

    w22pT = cload('w22pT', (70, C), F16)
    w11pnT = cload('w11pnT', (70, C))
    wm1stk = cload('wm1stk', (128, 128), F16)
    wm2stk = cload('wm2stk', (128, 128), F16)
    b22 = cload('b22', (C, 1))
    bqc = cload('bqc', (C, 1))
    bm1s = cload('bm1s', (128, 1))
    bm2s = cload('bm2s', (128, 1))
    id128 = cload('id128', (128, 128))
    cqidx = cload('cqidx', (128, 64 * NT), I16)
    eps64 = consts.tile([C, 1], F32, tag='eps64', name='eps64')
    nc.vector.memset(eps64[:], EPS)

    # ---------- per (batch, side) key prep ----------
    CH = 1024                      # normalization chunk width

    def key_khat(bi, side):
        # khat = knn / sqrt(colsum(knn^2) + eps), exact fp32 matching the
        # reference's rounding (sqrt of biased sum, then reciprocal —
        # reordering these flips near-tie neighbor selections).
        knn_d = ins['knn1'] if side == 1 else ins['knn2']
        knn = prep.tile([C, N], F32, tag='knn', name='knn', bufs=1)
        nc.sync.dma_start(knn[:], knn_d[bi])
        khat = keyp.tile([C, N], F32, tag='khat', name='khat')
        for j in range(N // CH):
            sl = slice(j * CH, (j + 1) * CH)
            ksq = prep.tile([C, CH], F32, tag='scrA', name='ksq')
            nc.gpsimd.tensor_tensor(ksq[:], knn[:, sl], knn[:, sl],
                                    op=ALU.mult)
            ssb = prep.tile([C, CH], F32, tag='scrB', name='ssb')
            nc.gpsimd.partition_all_reduce(ssb[:], ksq[:], channels=C,
                                           reduce_op=ROP.add)
            nc.scalar.activation(ssb[:], ssb[:], AF.Sqrt, bias=eps64[:])
            rinv = prep.tile([C, CH], F32, tag='scrA', name='rinv')
            nc.vector.reciprocal(rinv[:], ssb[:])
            nc.gpsimd.tensor_tensor(khat[:, sl], knn[:, sl], rinv[:],
                                    op=ALU.mult)
        return khat

    def key_rest(bi, side, khat):
        fkv_d = ins['fkv1'] if side == 1 else ins['fkv2']
        # fkv rows: 0-63 feat, 64-66 pc^2 (computed), 67-69 pc, 96-98 pc.
        fkv = inp.tile([FKR, N], F32, tag='fkv', name='fkv')
        nc.sync.dma_start(fkv[:], fkv_d[bi])
        nc.scalar.activation(fkv[C:C + 3, :], fkv[96:99, :], AF.Square)

        # akv2 = [W22; 0; Wpos] @ fkv[0:70] + b22, replicated to 128 parts
        fkv16 = f16p.tile([70, N], F16, tag='fkv16', name='fkv16')
        nc.gpsimd.tensor_copy(fkv16[:], fkv[0:70, :])
        akv2 = keyp.tile([128, N], F32, tag='akv2', name='akv2')
        for kb in range(N // 512):
            sl = slice(kb * 512, (kb + 1) * 512)
            ps = p64_ps.tile([C, 512], F32, tag='p64', name='akv_ps')
            nc.tensor.matmul(ps[:], lhsT=w22pT[:], rhs=fkv16[:, sl],
                             start=True, stop=True)
            nc.scalar.activation(akv2[0:C, sl], ps[:], AF.Identity,
                                 bias=b22[:])
        nc.sync.dma_start(akv2[C:128, :], akv2[0:C, :])
        return dict(fkv=fkv, khat=khat, akv2=akv2)

    def key_prep_batch(bi):
        kh1 = key_khat(bi, 1)
        kh2 = key_khat(bi, 2)
        return {1: key_rest(bi, 1, kh1), 2: key_rest(bi, 2, kh2)}

    # ---------- per-combo query prep ----------
    def query_prep(sd_q):
        fkv = sd_q['fkv']
        # cq = W11@feat_q - Wpos@pc_q + (b11 + bpos), stacked to 128 rows
        cq2 = qp.tile([128, Q], F32, tag='cq2', name='cq2')
        ps = p64_ps.tile([C, 512], F32, tag='p64', name='cq_ps')
        nc.tensor.matmul(ps[:, :Q], lhsT=w11pnT[:], rhs=fkv[0:70, 0:Q],
                         start=True, stop=True)
        nc.scalar.activation(cq2[0:C, :], ps[:, :Q], AF.Identity,
                             bias=bqc[:])
        nc.scalar.activation(cq2[C:128, :], ps[:, :Q], AF.Identity,
                             bias=bqc[:])
        # augq rows 64-69: [-0.5 x3, qx, qy, qz] — contracts with fkv rows
        # 64-69 = [x^2, y^2, z^2, x, y, z]: score = q.k - 0.5|k|^2.
        augq = qp.tile([70, Q], F32, tag='augq', name='augq')
        nc.scalar.activation(augq[C:C + 3, :], fkv[C:C + 3, 0:Q],
                             AF.Copy, scale=0.0, bias=-0.5)
        nc.sync.dma_start(augq[C + 3:C + 6, :], fkv[C + 3:C + 6, 0:Q])
        return dict(cq2=cq2, augq=augq)

    # ---------- tile stages ----------
    def tile_scores(cb):
        sd_q, sd_k, t = cb['q'], cb['k'], cb['t']
        tsl = slice(t * 128, (t + 1) * 128)
        khat_q, khat_k = sd_q['khat'], sd_k['khat']
        fkv_k = sd_k['fkv']
        augq = cb['qd']['augq']

        sc_cos = scp.tile([128, N], F32, tag='sc', name='sc_cos')
        for j in range(N // 1024):
            ps = sc_ps.tile([128, 1024], F32, tag='sc_ps', name='sc_ps')
            for h in range(2):
                sl = slice(j * 1024 + h * 512, j * 1024 + (h + 1) * 512)
                nc.tensor.matmul(ps[:, h * 512:(h + 1) * 512],
                                 lhsT=khat_q[:, tsl], rhs=khat_k[:, sl],
                                 start=True, stop=True)
            nc.scalar.activation(sc_cos[:, j * 1024:(j + 1) * 1024],
                                 ps[:], AF.Copy)
        sc_euc = scp.tile([128, N], F32, tag='sc', name='sc_euc')
        for j in range(N // 1024):
            ps = sc_ps.tile([128, 1024], F32, tag='sc_ps', name='sc_ps')
            for h in range(2):
                sl = slice(j * 1024 + h * 512, j * 1024 + (h + 1) * 512)
                nc.tensor.matmul(ps[:, h * 512:(h + 1) * 512],
                                 lhsT=augq[C:C + 6, tsl],
                                 rhs=fkv_k[C:C + 6, sl],
                                 start=True, stop=True)
            nc.scalar.activation(sc_euc[:, j * 1024:(j + 1) * 1024],
                                 ps[:], AF.Copy)
        cb['sc_cos'], cb['sc_euc'] = sc_cos, sc_euc
        if taps is not None and cb['ci'] == 0 and t == 0:
            nc.sync.dma_start(taps['dbg_khat'], sd_k['khat'][:])
            nc.sync.dma_start(taps['dbg_fkv'], fkv_k[:])
            nc.sync.dma_start(taps['dbg_akv2'], sd_k['akv2'][:])
            nc.sync.dma_start(taps['dbg_cq2'], cb['qd']['cq2'][:])
            nc.sync.dma_start(taps['dbg_augq'], augq[:])
            nc.sync.dma_start(taps['dbg_sccos'], sc_cos[:])
            nc.sync.dma_start(taps['dbg_sceuc'], sc_euc[:])

    def tile_topk(cb):
        vals = idxp.tile([128, 16], F32, tag='vals', name='vals')
        idxu = idxp.tile([128, 16], U16, tag='idxu', name='idxu')
        nc.vector.max(vals[:, 0:8], cb['sc_cos'][:])
        nc.vector.max_index(idxu[:, 0:8], vals[:, 0:8], cb['sc_cos'][:])
        nc.vector.max(vals[:, 8:16], cb['sc_euc'][:])
        nc.vector.max_index(idxu[:, 8:16], vals[:, 8:16], cb['sc_euc'][:])
        cb['idxu'] = idxu

    def tile_post(cb):
        sd_k, t = cb['k'], cb['t']
        cq2 = cb['qd']['cq2']
        akv2 = sd_k['akv2']

        # ---- index transpose to gather layout ----
        idxf = idxp.tile([128, 16], F32, tag='idxf', name='idxf')
        nc.vector.tensor_copy(idxf[:], cb['idxu'][:])
        pst = tp_ps.tile([16, 128], F32, tag='tp', name='pst')
        nc.tensor.matmul(pst[:], lhsT=idxf[:], rhs=id128[:],
                         start=True, stop=True)
        idxrow = idxp.tile([16, 128], I16, tag='idxrow', name='idxrow')
        nc.scalar.activation(idxrow[:], pst[:], AF.Copy)
        idxT = idxp.tile([128, 64], I16, tag='idxT', name='idxT')
        for h in range(2):
            b = h * 64
            nc.sync.dma_start(idxT[b:b + 16, :],
                              idxrow[:, h * 64:(h + 1) * 64])
            nc.sync.dma_start(idxT[b + 16:b + 32, :], idxT[b:b + 16, :])
            nc.sync.dma_start(idxT[b + 32:b + 64, :], idxT[b:b + 32, :])

        # ---- gathers: neighbors + per-query cq expansion ----
        ag = mlpp.tile([128, 1024], F32, tag='ag', name='ag', bufs=2)
        nc.gpsimd.ap_gather(ag[:], akv2[:], idxT[:], channels=128,
                            num_elems=N, d=1, num_idxs=1024)
        cqs = mlpp.tile([128, 1024], F32, tag='cqs', name='cqs')
        nc.gpsimd.ap_gather(cqs[:], cq2[:], cqidx[:, t * 64:(t + 1) * 64],
                            channels=128, num_elems=Q, d=1, num_idxs=1024)
        if taps is not None and cb['ci'] == 0 and t == 0:
            nc.sync.dma_start(taps['dbg_idxf'], idxf[:])
            nc.sync.dma_start(taps['dbg_ag'], ag[:])
            nc.sync.dma_start(taps['dbg_cqs'], cqs[:])

        # ---- layer 0: add cq, leaky (-> fp16) ----
        nc.gpsimd.tensor_tensor(ag[:], ag[:], cqs[:], op=ALU.add)
        n1 = mlpp.tile([128, 1024], F16, tag='n1', name='n1')
        nc.scalar.activation(n1[:], ag[:], AF.Prelu, alpha=LEAKY)

        # ---- layers 1, 2 (fp16, stacked weights) ----
        cur = n1
        for li, (w, bias) in enumerate(((wm1stk, bm1s), (wm2stk, bm2s))):
            ps = mlp_ps.tile([128, 1024], F32, tag='mlp', name='mlp_ps')
            for h in range(2):
                hs = slice(h * 512, (h + 1) * 512)
                nc.tensor.matmul(ps[:, hs], lhsT=w[:], rhs=cur[:, hs],
                                 start=True, stop=True)
            ht = mlpp.tile([128, 1024], F16, tag=f'h{li + 1}',
                           name=f'h{li + 1}')
            nc.scalar.activation(ht[:], ps[:], AF.Prelu, bias=bias[:],
                                 alpha=LEAKY)
            cur = ht
        if taps is not None and cb['ci'] == 0 and t == 0:
            nc.sync.dma_start(taps['dbg_h2'], cur[:])

        # ---- maxpool over NS neighbors (DVE half-pair tree: packed
        # contiguous slot runs keep the fp16 2x mode) ----
        width = NS
        while width > 1:
            w2 = width // 2
            if w2 == 1:
                nxt = cb['outc']
                dst = nxt[:, t * 64:(t + 1) * 64].rearrange(
                    'c (q k) -> c q k', k=1)
            else:
                nxt = mlpp.tile([128, 64 * w2], F16, tag=f'mp{w2}',
                                name=f'mp{w2}')
                dst = nxt[:].rearrange('c (q k) -> c q k', k=w2)
            v = cur[:].rearrange('c (q k) -> c q k', k=width)
            nc.vector.tensor_tensor(dst, v[:, :, 0:w2], v[:, :, w2:width],
                                    op=ALU.max)
            cur = nxt
            width = w2

    def emit_out(cb):
        # one contiguous DMA per combo: [128, 256] -> out[b, h, c, :]
        outap = cb['outap']
        base = outap.offset + cb['bi'] * 2 * C * 64 * NT
        dst = bass.AP(outap.tensor, base,
                      [[C * 64 * NT, 2], [64 * NT, C], [1, 64 * NT]])
        nc.sync.dma_start(dst, cb['outc'][:])

    # ---------- main loop (software pipelined, incl. batch-level) ----------
    pending = None
    pending_out = None
    ci = 0
    sd_next = key_prep_batch(0)
    for bi in range(B):
        sd = sd_next
        for outap, qs, ks in ((out1, 1, 2), (out2, 2, 1)):
            qd = query_prep(sd[qs])
            outc = outp.tile([128, 64 * NT], F32, tag='outc', name='outc')
            for t in range(NT):
                cb = dict(q=sd[qs], k=sd[ks], qd=qd, t=t, bi=bi,
                          outap=outap, outc=outc, ci=ci)
                tile_scores(cb)
                if pending is not None:
                    tile_post(pending)
                if pending_out is not None and ci > pending_out[0] \
                        and t == 1:
                    emit_out(pending_out[1])
                    pending_out = None
                tile_topk(cb)
                pending = cb
            pending_out = (ci, cb)
            ci += 1
        if bi + 1 < B:
            # prefetch next batch's key prep behind this batch's tail
            sd_next = key_prep_batch(bi + 1)
    tile_post(pending)
    emit_out(pending_out[1])
    ctx.close()


# ======================= host side =======================

_CACHED = {}


def _get_nc():
    if 'nc' not in _CACHED:
        _CACHED['nc'] = build_nc()
    return _CACHED['nc']


def make_in_maps(pc1, pc2, feat1, feat2, knn1, knn2,
                 W_t11, b_t11, W_t22, b_t22, W_pos, b_pos,
                 W_m1, b_m1, W_m2, b_m2):
    f32, f16 = np.float32, np.float16
    W_t11 = np.asarray(W_t11, f32); W_t22 = np.asarray(W_t22, f32)
    W_pos = np.asarray(W_pos, f32)
    W_m1 = np.asarray(W_m1, f32); W_m2 = np.asarray(W_m2, f32)

    z3 = np.zeros((3, C), f32)
    w22pT = np.vstack([W_t22.T, z3, W_pos.T]).astype(f16)        # [70, 64]
    w11pnT = np.vstack([W_t11.T, z3, -W_pos.T]).astype(f32)      # [70, 64]
    z = np.zeros((C, C), f32)
    wm1stk = np.block([[W_m1.T, z], [z, W_m1.T]]).astype(f16)    # [128,128]
    wm2stk = np.block([[W_m2.T, z], [z, W_m2.T]]).astype(f16)
    b22 = np.asarray(b_t22, f32).reshape(C, 1)
    bqc = (np.asarray(b_t11, f32) + np.asarray(b_pos, f32)).reshape(C, 1)
    bm1s = np.tile(np.asarray(b_m1, f32).reshape(C, 1), (2, 1))
    bm2s = np.tile(np.asarray(b_m2, f32).reshape(C, 1), (2, 1))

    # cqidx[16g+s, t*64+q] = t*128 + 64*(g>=4) + q: groups 0-3 expand the
    # first 64 queries of tile t, groups 4-7 the second 64.
    cqidx = np.zeros((128, 64 * NT), np.int16)
    for g in range(8):
        h = g // 4
        for t in range(NT):
            cqidx[16 * g:16 * (g + 1), t * 64:(t + 1) * 64] = \
                t * 128 + 64 * h + np.arange(64, dtype=np.int16)[None, :]

    def build_fkv(feat, pc):
        b, _, n = feat.shape
        fkv = np.zeros((b, FKR, n), f32)
        fkv[:, 0:C] = feat
        fkv[:, C + 3:C + 6] = pc
        fkv[:, 96:99] = pc
        return fkv

    fkv1 = build_fkv(np.asarray(feat1, f32), np.asarray(pc1, f32))
    fkv2 = build_fkv(np.asarray(feat2, f32), np.asarray(pc2, f32))
    knn1 = np.asarray(knn1, f32)
    knn2 = np.asarray(knn2, f32)

    base = {
        'w22pT': w22pT, 'w11pnT': w11pnT,
        'wm1stk': wm1stk, 'wm2stk': wm2stk,
        'b22': b22, 'bqc': bqc, 'bm1s': bm1s, 'bm2s': bm2s,
        'id128': np.eye(128, dtype=f32),
        'cqidx': cqidx,
    }
    in_maps = []
    for c in range(NCORES):
        m = dict(base)
        r = -c * Q
        m['knn1'] = np.ascontiguousarray(np.roll(knn1, r, axis=2))
        m['knn2'] = np.ascontiguousarray(np.roll(knn2, r, axis=2))
        m['fkv1'] = np.ascontiguousarray(np.roll(fkv1, r, axis=2))
        m['fkv2'] = np.ascontiguousarray(np.roll(fkv2, r, axis=2))
        in_maps.append(m)
    return in_maps


def _unstack_out(res, name):
    # per-core out [B, 2, C, 256] -> [B, C, 512] block, concat on queries
    blocks = []
    for c in range(NCORES):
        o = res.results[c][name]           # [B, 2, C, 256]
        o = o.reshape(B, 2, C, NT, 64)     # [b, h, c, t, q]
        o = o.transpose(0, 2, 3, 1, 4)     # [b, c, t, h, q]
        blocks.append(o.reshape(B, C, Q))
    return np.concatenate(blocks, axis=2)


def kernel(pc1, pc2, feat1, feat2, knn1, knn2,
           W_t11, b_t11, W_t22, b_t22, W_pos, b_pos,
           W_m1, b_m1, W_m2, b_m2):
    from concourse.bass_utils import run_bass_kernel_spmd
    nc = _get_nc()
    in_maps = make_in_maps(pc1, pc2, feat1, feat2, knn1, knn2,
                           W_t11, b_t11, W_t22, b_t22, W_pos, b_pos,
                           W_m1, b_m1, W_m2, b_m2)
    res = run_bass_kernel_spmd(nc, in_maps, core_ids=list(range(NCORES)))
    return _unstack_out(res, 'out1'), _unstack_out(res, 'out2')


# revision 70
# speedup vs baseline: 63.6038x; 1.0593x over previous
"""Trainium2 Bass kernel for nn_BidirectionalLayerFeatCosine (retrieval_knn).

Strategy: shard the 4096 query points across 8 NeuronCores (512 each); keys
are replicated.  Host rolls the key axis per core so each core's query block
is always columns 0:512 (SPMD-clean static slices).

Per core, per batch, per side: ONE wide DMA loads [feat; _; pc; pc-dup]
(fkv tile, rows 0-63 / 64-66(pc^2 target) / 67-69 / 96-98); knn is loaded in
chunks and normalized exactly in fp32 via gpsimd partition_all_reduce + ACT
sqrt + DVE reciprocal + gpsimd multiply -> khat (query side is a slice).
akv = W22@feat + Wpos@pc + b22 via one fp16 70-row matmul per chunk,
replicated to 128 partitions for the stacked gather.  Scores (cos + euclid)
stay exact fp32 on the PE; top-8 via DVE max8/find_index8; ap_gather pulls
neighbors (and a second static-index gather expands cq); the fp16 MLP uses
block-diagonal stacked weights; maxpool pair-tree; per-combo output
accumulation with one contiguous DMA emitted a combo late.
"""
import sys

for _p in ('/opt/trn_rl_repo',):
    if _p not in sys.path:
        sys.path.insert(0, _p)

import numpy as np
import concourse.bass as bass
import concourse.tile as tile
from concourse import bacc, mybir, bass_isa

F32 = mybir.dt.float32
F16 = mybir.dt.float16
I16 = mybir.dt.int16
U16 = mybir.dt.uint16
AF = mybir.ActivationFunctionType
ALU = mybir.AluOpType
ROP = bass_isa.ReduceOp

B, N, C, NS = 2, 4096, 64, 16
NCORES = 8
Q = N // NCORES           # queries per core per combo (512)
NT = Q // 128             # query tiles per combo (4)
LEAKY = 0.1
EPS = 1e-8
FKR = 99                  # fkv tile rows


def build_nc(debug_taps=False):
    nc = bacc.Bacc("TRN2", num_devices=NCORES, debug=False)

    def din(name, shape, dt=F32):
        return nc.dram_tensor(name, list(shape), dt, kind="ExternalInput").ap()

    ins = {
        'knn12': din('knn12', (B, C, 2 * N)),
        'fkv1': din('fkv1', (B, FKR, N)),
        'fkv2': din('fkv2', (B, FKR, N)),
        'w22pT': din('w22pT', (70, C), F16),
        'w11pnT': din('w11pnT', (70, C)),
        'wm1stk': din('wm1stk', (128, 128), F16),
        'wm2stk': din('wm2stk', (128, 128), F16),
        'b22': din('b22', (C, 1)),
        'bqc': din('bqc', (C, 1)),
        'bm1s': din('bm1s', (128, 1)),
        'bm2s': din('bm2s', (128, 1)),
        'id128': din('id128', (128, 128)),
        'cqidx': din('cqidx', (128, 64 * NT), I16),
    }
    # out[b, h, c, t*64+q] = feat_new[b, c, 512*core + t*128 + 64*h + q]
    out1 = nc.dram_tensor('out1', [B, 2, C, 64 * NT], F32,
                          kind="ExternalOutput").ap()
    out2 = nc.dram_tensor('out2', [B, 2, C, 64 * NT], F32,
                          kind="ExternalOutput").ap()
    taps = None
    if debug_taps:
        taps = {nm: nc.dram_tensor(nm, list(sh), dt, kind="ExternalOutput").ap()
                for nm, sh, dt in [
                    ('dbg_khat', (C, N), F32),
                    ('dbg_fkv', (FKR, N), F32),
                    ('dbg_akv2', (128, N), F32),
                    ('dbg_cq2', (128, Q), F32),
                    ('dbg_augq', (70, Q), F32),
                    ('dbg_sccos', (128, N), F32),
                    ('dbg_sceuc', (128, N), F32),
                    ('dbg_idxf', (128, 16), F32),
                    ('dbg_ag', (128, 1024), F32),
                    ('dbg_cqs', (128, 1024), F32),
                    ('dbg_h2', (128, 1024), F16),
                ]}

    with tile.TileContext(nc) as tc:
        _body(tc, ins, out1, out2, taps)
    nc.compile()
    return nc


def _body(tc, ins, out1, out2, taps=None):
    nc = tc.nc
    from contextlib import ExitStack
    ctx = ExitStack()

    pool = lambda name, bufs, space='SBUF': ctx.enter_context(
        tc.tile_pool(name=name, bufs=bufs, space=space))

    consts = pool('consts', 1)
    inp = pool('inputs', 2)        # fkv tiles (rotate across sides/batches)
    prep = pool('prep', 2)         # chunked scratch for normalization
    keyp = pool('keyprep', 2)      # khat / akv2 (both sides live)
    f16p = pool('f16', 1)
    qp = pool('qside', 2)          # cq2 / augq per combo
    scp = pool('scores', 2)        # [128, 4096] score rows
    idxp = pool('idx', 2)          # vals/idx tiles per tile
    mlpp = pool('mlp', 1)
    outp = pool('out', 2)

    sc_ps = ctx.enter_context(tc.tile_pool(name='sc_ps', bufs=2, space='PSUM'))
    mlp_ps = ctx.enter_context(tc.tile_pool(name='mlp_ps', bufs=1, space='PSUM'))
    p64_ps = ctx.enter_context(tc.tile_pool(name='p64_ps', bufs=1, space='PSUM'))
    tp_ps = ctx.enter_context(tc.tile_pool(name='tp_ps', bufs=1, space='PSUM'))

    # ---- constants ----
    def cload(name, shape, dt=F32):
        t = consts.tile(list(shape), dt, tag=name, name=name)
        nc.sync.dma_start(t[:], ins[name])
        return t

    w22pT = cload('w22pT', (70, C), F16)
    w11pnT = cload('w11pnT', (70, C))
    wm1stk = cload('wm1stk', (128, 128), F16)
    wm2stk = cload('wm2stk', (128, 128), F16)
    b22 = cload('b22', (C, 1))
    bqc = cload('bqc', (C, 1))
    bm1s = cload('bm1s', (128, 1))
    bm2s = cload('bm2s', (128, 1))
    id128 = cload('id128', (128, 128))
    cqidx = cload('cqidx', (128, 64 * NT), I16)
    eps128 = consts.tile([128, 1], F32, tag='eps128', name='eps128')
    nc.vector.memset(eps128[:], EPS)

    # ---------- per (batch, side) key prep ----------
    CH = 1024                      # normalization chunk width

    def key_khat_pair(bi):
        # khat = knn / sqrt(colsum(knn^2) + eps), exact fp32 matching the
        # reference's rounding (sqrt of biased sum, then reciprocal —
        # reordering these flips near-tie neighbor selections).  Both sides
        # column-stacked [64, 2N]: one DMA, one uniform 8-chunk chain, and
        # score matmuls slice either side at base partition 0.
        knn = prep.tile([C, 2 * N], F32, tag='knn', name='knn', bufs=1)
        nc.sync.dma_start(knn[:], ins['knn12'][bi])
        khatC = keyp.tile([C, 2 * N], F32, tag='khatC', name='khatC',
                          bufs=1)
        for j in range(2 * N // CH):
            sl = slice(j * CH, (j + 1) * CH)
            ksq = prep.tile([C, CH], F32, tag='scrA', name='ksq')
            nc.gpsimd.tensor_tensor(ksq[:], knn[:, sl], knn[:, sl],
                                    op=ALU.mult)
            ssb = prep.tile([C, CH], F32, tag='scrB', name='ssb')
            nc.gpsimd.partition_all_reduce(ssb[:], ksq[:], channels=C,
                                           reduce_op=ROP.add)
            nc.scalar.activation(ssb[:], ssb[:], AF.Sqrt, bias=eps128[0:C])
            rinv = prep.tile([C, CH], F32, tag='scrA', name='rinv')
            nc.vector.reciprocal(rinv[:], ssb[:])
            nc.gpsimd.tensor_tensor(khatC[:, sl], knn[:, sl], rinv[:],
                                    op=ALU.mult)
        return khatC

    def key_rest(bi, side, khat):
        fkv_d = ins['fkv1'] if side == 1 else ins['fkv2']
        # fkv rows: 0-63 feat, 64-66 pc^2 (computed), 67-69 pc, 96-98 pc.
        fkv = inp.tile([FKR, N], F32, tag='fkv', name='fkv')
        nc.sync.dma_start(fkv[:], fkv_d[bi])
        nc.scalar.activation(fkv[C:C + 3, :], fkv[96:99, :], AF.Square)

        # akv2 = [W22; 0; Wpos] @ fkv[0:70] + b22, replicated to 128 parts
        akv2 = keyp.tile([128, N], F32, tag='akv2', name='akv2')
        for kb in range(N // 512):
            sl = slice(kb * 512, (kb + 1) * 512)
            fkv16 = f16p.tile([70, 512], F16, tag='fkv16', name='fkv16',
                              bufs=2)
            nc.gpsimd.tensor_copy(fkv16[:], fkv[0:70, sl])
            ps = p64_ps.tile([C, 512], F32, tag='p64', name='akv_ps')
            nc.tensor.matmul(ps[:], lhsT=w22pT[:], rhs=fkv16[:],
                             start=True, stop=True)
            nc.scalar.activation(akv2[0:C, sl], ps[:], AF.Identity,
                                 bias=b22[:])
        nc.sync.dma_start(akv2[C:128, :], akv2[0:C, :])
        return dict(fkv=fkv, khat=khat, akv2=akv2)

    def key_prep_batch(bi):
        khatC = key_khat_pair(bi)
        sd = {1: key_rest(bi, 1, khatC), 2: key_rest(bi, 2, khatC)}
        sd[1]['koff'], sd[2]['koff'] = 0, N
        return sd

    # ---------- per-combo query prep ----------
    def query_prep(sd_q):
        fkv = sd_q['fkv']
        # cq = W11@feat_q - Wpos@pc_q + (b11 + bpos), stacked to 128 rows
        cq2 = qp.tile([128, Q], F32, tag='cq2', name='cq2')
        ps = p64_ps.tile([C, 512], F32, tag='p64', name='cq_ps')
        nc.tensor.matmul(ps[:, :Q], lhsT=w11pnT[:], rhs=fkv[0:70, 0:Q],
                         start=True, stop=True)
        nc.scalar.activation(cq2[0:C, :], ps[:, :Q], AF.Identity,
                             bias=bqc[:])
        nc.scalar.activation(cq2[C:128, :], ps[:, :Q], AF.Identity,
                             bias=bqc[:])
        # augq rows 64-69: [-0.5 x3, qx, qy, qz] — contracts with fkv rows
        # 64-69 = [x^2, y^2, z^2, x, y, z]: score = q.k - 0.5|k|^2.
        augq = qp.tile([70, Q], F32, tag='augq', name='augq')
        nc.scalar.activation(augq[C:C + 3, :], fkv[C:C + 3, 0:Q],
                             AF.Copy, scale=0.0, bias=-0.5)
        nc.sync.dma_start(augq[C + 3:C + 6, :], fkv[C + 3:C + 6, 0:Q])
        return dict(cq2=cq2, augq=augq)

    # ---------- tile stages ----------
    def tile_scores(cb):
        sd_q, sd_k, t = cb['q'], cb['k'], cb['t']
        tsl = slice(t * 128, (t + 1) * 128)
        qtsl = slice(sd_q['koff'] + t * 128, sd_q['koff'] + (t + 1) * 128)
        khat = sd_q['khat']
        ko = sd_k['koff']
        fkv_k = sd_k['fkv']
        augq = cb['qd']['augq']

        sc_cos = scp.tile([128, N], F32, tag='sc', name='sc_cos')
        for j in range(N // 1024):
            ps = sc_ps.tile([128, 1024], F32, tag='sc_ps', name='sc_ps')
            for h in range(2):
                sl = slice(ko + j * 1024 + h * 512,
                           ko + j * 1024 + (h + 1) * 512)
                nc.tensor.matmul(ps[:, h * 512:(h + 1) * 512],
                                 lhsT=khat[:, qtsl], rhs=khat[:, sl],
                                 start=True, stop=True)
            nc.scalar.activation(sc_cos[:, j * 1024:(j + 1) * 1024],
                                 ps[:], AF.Copy)
        sc_euc = scp.tile([128, N], F32, tag='sc', name='sc_euc')
        for j in range(N // 1024):
            ps = sc_ps.tile([128, 1024], F32, tag='sc_ps', name='sc_ps')
            for h in range(2):
                sl = slice(j * 1024 + h * 512, j * 1024 + (h + 1) * 512)
                nc.tensor.matmul(ps[:, h * 512:(h + 1) * 512],
                                 lhsT=augq[C:C + 6, tsl],
                                 rhs=fkv_k[C:C + 6, sl],
                                 start=True, stop=True)
            nc.scalar.activation(sc_euc[:, j * 1024:(j + 1) * 1024],
                                 ps[:], AF.Copy)
        cb['sc_cos'], cb['sc_euc'] = sc_cos, sc_euc
        if taps is not None and cb['ci'] == 0 and t == 0:
            nc.sync.dma_start(taps['dbg_khat'], khat[:, ko:ko + N])
            nc.sync.dma_start(taps['dbg_fkv'], fkv_k[:])
            nc.sync.dma_start(taps['dbg_akv2'], sd_k['akv2'][:])
            nc.sync.dma_start(taps['dbg_cq2'], cb['qd']['cq2'][:])
            nc.sync.dma_start(taps['dbg_augq'], augq[:])
            nc.sync.dma_start(taps['dbg_sccos'], sc_cos[:])
            nc.sync.dma_start(taps['dbg_sceuc'], sc_euc[:])

    def tile_topk(cb):
        vals = idxp.tile([128, 16], F32, tag='vals', name='vals')
        idxu = idxp.tile([128, 16], U16, tag='idxu', name='idxu')
        nc.vector.max(vals[:, 0:8], cb['sc_cos'][:])
        nc.vector.max_index(idxu[:, 0:8], vals[:, 0:8], cb['sc_cos'][:])
        nc.vector.max(vals[:, 8:16], cb['sc_euc'][:])
        nc.vector.max_index(idxu[:, 8:16], vals[:, 8:16], cb['sc_euc'][:])
        cb['idxu'] = idxu

    def tile_post(cb):
        sd_k, t = cb['k'], cb['t']
        cq2 = cb['qd']['cq2']
        akv2 = sd_k['akv2']

        # ---- index transpose to gather layout ----
        idxf = idxp.tile([128, 16], F32, tag='idxf', name='idxf')
        nc.vector.tensor_copy(idxf[:], cb['idxu'][:])
        pst = tp_ps.tile([16, 128], F32, tag='tp', name='pst')
        nc.tensor.matmul(pst[:], lhsT=idxf[:], rhs=id128[:],
                         start=True, stop=True)
        idxrow = idxp.tile([16, 128], I16, tag='idxrow', name='idxrow')
        nc.scalar.activation(idxrow[:], pst[:], AF.Copy)
        idxT = idxp.tile([128, 64], I16, tag='idxT', name='idxT')
        for h in range(2):
            b = h * 64
            nc.sync.dma_start(idxT[b:b + 16, :],
                              idxrow[:, h * 64:(h + 1) * 64])
            nc.sync.dma_start(idxT[b + 16:b + 32, :], idxT[b:b + 16, :])
            nc.sync.dma_start(idxT[b + 32:b + 64, :], idxT[b:b + 32, :])

        # ---- gathers: neighbors + per-query cq expansion ----
        ag = mlpp.tile([128, 1024], F32, tag='ag', name='ag')
        nc.gpsimd.ap_gather(ag[:], akv2[:], idxT[:], channels=128,
                            num_elems=N, d=1, num_idxs=1024)
        cqs = mlpp.tile([128, 1024], F32, tag='cqs', name='cqs')
        nc.gpsimd.ap_gather(cqs[:], cq2[:], cqidx[:, t * 64:(t + 1) * 64],
                            channels=128, num_elems=Q, d=1, num_idxs=1024)
        if taps is not None and cb['ci'] == 0 and t == 0:
            nc.sync.dma_start(taps['dbg_idxf'], idxf[:])
            nc.sync.dma_start(taps['dbg_ag'], ag[:])
            nc.sync.dma_start(taps['dbg_cqs'], cqs[:])

        # ---- layer 0: add cq, leaky (-> fp16) ----
        nc.gpsimd.tensor_tensor(ag[:], ag[:], cqs[:], op=ALU.add)
        n1 = mlpp.tile([128, 1024], F16, tag='n1', name='n1')
        nc.scalar.activation(n1[:], ag[:], AF.Prelu, alpha=LEAKY)

        # ---- layers 1, 2 (fp16, stacked weights) ----
        cur = n1
        for li, (w, bias) in enumerate(((wm1stk, bm1s), (wm2stk, bm2s))):
            ps = mlp_ps.tile([128, 1024], F32, tag='mlp', name='mlp_ps')
            for h in range(2):
                hs = slice(h * 512, (h + 1) * 512)
                nc.tensor.matmul(ps[:, hs], lhsT=w[:], rhs=cur[:, hs],
                                 start=True, stop=True)
            ht = mlpp.tile([128, 1024], F16, tag=f'h{li + 1}',
                           name=f'h{li + 1}')
            nc.scalar.activation(ht[:], ps[:], AF.Prelu, bias=bias[:],
                                 alpha=LEAKY)
            cur = ht
        if taps is not None and cb['ci'] == 0 and t == 0:
            nc.sync.dma_start(taps['dbg_h2'], cur[:])

        # ---- maxpool over NS neighbors (DVE half-pair tree: packed
        # contiguous slot runs keep the fp16 2x mode) ----
        width = NS
        while width > 1:
            w2 = width // 2
            if w2 == 1:
                nxt = cb['outc']
                dst = nxt[:, t * 64:(t + 1) * 64].rearrange(
                    'c (q k) -> c q k', k=1)
            else:
                nxt = mlpp.tile([128, 64 * w2], F16, tag=f'mp{w2}',
                                name=f'mp{w2}')
                dst = nxt[:].rearrange('c (q k) -> c q k', k=w2)
            v = cur[:].rearrange('c (q k) -> c q k', k=width)
            nc.vector.tensor_tensor(dst, v[:, :, 0:w2], v[:, :, w2:width],
                                    op=ALU.max)
            cur = nxt
            width = w2

    def emit_out(cb):
        # one contiguous DMA per combo: [128, 256] -> out[b, h, c, :]
        outap = cb['outap']
        base = outap.offset + cb['bi'] * 2 * C * 64 * NT
        dst = bass.AP(outap.tensor, base,
                      [[C * 64 * NT, 2], [64 * NT, C], [1, 64 * NT]])
        nc.sync.dma_start(dst, cb['outc'][:])

    # ---------- main loop (software pipelined, incl. batch-level) ----------
    pending = None
    pending_out = None
    ci = 0
    sd_next = key_prep_batch(0)
    for bi in range(B):
        sd = sd_next
        for outap, qs, ks in ((out1, 1, 2), (out2, 2, 1)):
            qd = query_prep(sd[qs])
            outc = outp.tile([128, 64 * NT], F32, tag='outc', name='outc')
            for t in range(NT):
                cb = dict(q=sd[qs], k=sd[ks], qd=qd, t=t, bi=bi,
                          outap=outap, outc=outc, ci=ci)
                tile_scores(cb)
                if pending is not None:
                    tile_post(pending)
                if pending_out is not None and ci > pending_out[0] \
                        and t == 1:
                    emit_out(pending_out[1])
                    pending_out = None
                tile_topk(cb)
                pending = cb
            pending_out = (ci, cb)
            ci += 1
        if bi + 1 < B:
            # prefetch next batch's key prep behind this batch's tail
            sd_next = key_prep_batch(bi + 1)
    tile_post(pending)
    emit_out(pending_out[1])
    ctx.close()


# ======================= host side =======================

_CACHED = {}


def _get_nc():
    if 'nc' not in _CACHED:
        _CACHED['nc'] = build_nc()
    return _CACHED['nc']


def make_in_maps(pc1, pc2, feat1, feat2, knn1, knn2,
                 W_t11, b_t11, W_t22, b_t22, W_pos, b_pos,
                 W_m1, b_m1, W_m2, b_m2):
    f32, f16 = np.float32, np.float16
    W_t11 = np.asarray(W_t11, f32); W_t22 = np.asarray(W_t22, f32)
    W_pos = np.asarray(W_pos, f32)
    W_m1 = np.asarray(W_m1, f32); W_m2 = np.asarray(W_m2, f32)

    z3 = np.zeros((3, C), f32)
    w22pT = np.vstack([W_t22.T, z3, W_pos.T]).astype(f16)        # [70, 64]
    w11pnT = np.vstack([W_t11.T, z3, -W_pos.T]).astype(f32)      # [70, 64]
    z = np.zeros((C, C), f32)
    wm1stk = np.block([[W_m1.T, z], [z, W_m1.T]]).astype(f16)    # [128,128]
    wm2stk = np.block([[W_m2.T, z], [z, W_m2.T]]).astype(f16)
    b22 = np.asarray(b_t22, f32).reshape(C, 1)
    bqc = (np.asarray(b_t11, f32) + np.asarray(b_pos, f32)).reshape(C, 1)
    bm1s = np.tile(np.asarray(b_m1, f32).reshape(C, 1), (2, 1))
    bm2s = np.tile(np.asarray(b_m2, f32).reshape(C, 1), (2, 1))

    # cqidx[16g+s, t*64+q] = t*128 + 64*(g>=4) + q: groups 0-3 expand the
    # first 64 queries of tile t, groups 4-7 the second 64.
    cqidx = np.zeros((128, 64 * NT), np.int16)
    for g in range(8):
        h = g // 4
        for t in range(NT):
            cqidx[16 * g:16 * (g + 1), t * 64:(t + 1) * 64] = \
                t * 128 + 64 * h + np.arange(64, dtype=np.int16)[None, :]

    def build_fkv(feat, pc):
        b, _, n = feat.shape
        fkv = np.zeros((b, FKR, n), f32)
        fkv[:, 0:C] = feat
        fkv[:, C + 3:C + 6] = pc
        fkv[:, 96:99] = pc
        return fkv

    fkv1 = build_fkv(np.asarray(feat1, f32), np.asarray(pc1, f32))
    fkv2 = build_fkv(np.asarray(feat2, f32), np.asarray(pc2, f32))
    knn1 = np.asarray(knn1, f32)
    knn2 = np.asarray(knn2, f32)

    base = {
        'w22pT': w22pT, 'w11pnT': w11pnT,
        'wm1stk': wm1stk, 'wm2stk': wm2stk,
        'b22': b22, 'bqc': bqc, 'bm1s': bm1s, 'bm2s': bm2s,
        'id128': np.eye(128, dtype=f32),
        'cqidx': cqidx,
    }
    in_maps = []
    for c in range(NCORES):
        m = dict(base)
        r = -c * Q
        m['knn12'] = np.ascontiguousarray(np.concatenate(
            [np.roll(knn1, r, axis=2), np.roll(knn2, r, axis=2)], axis=2))
        m['fkv1'] = np.ascontiguousarray(np.roll(fkv1, r, axis=2))
        m['fkv2'] = np.ascontiguousarray(np.roll(fkv2, r, axis=2))
        in_maps.append(m)
    return in_maps


def _unstack_out(res, name):
    # per-core out [B, 2, C, 256] -> [B, C, 512] block, concat on queries
    blocks = []
    for c in range(NCORES):
        o = res.results[c][name]           # [B, 2, C, 256]
        o = o.reshape(B, 2, C, NT, 64)     # [b, h, c, t, q]
        o = o.transpose(0, 2, 3, 1, 4)     # [b, c, t, h, q]
        blocks.append(o.reshape(B, C, Q))
    return np.concatenate(blocks, axis=2)


def kernel(pc1, pc2, feat1, feat2, knn1, knn2,
           W_t11, b_t11, W_t22, b_t22, W_pos, b_pos,
           W_m1, b_m1, W_m2, b_m2):
    from concourse.bass_utils import run_bass_kernel_spmd
    nc = _get_nc()
    in_maps = make_in_maps(pc1, pc2, feat1, feat2, knn1, knn2,
                           W_t11, b_t11, W_t22, b_t22, W_pos, b_pos,
                           W_m1, b_m1, W_m2, b_m2)
    res = run_bass_kernel_spmd(nc, in_maps, core_ids=list(range(NCORES)))
    return _unstack_out(res, 'out1'), _unstack_out(res, 'out2')
